# revision 70
# baseline (speedup 1.0000x reference)
"""Trainium2 Bass kernel for nn_DocREModel (DocRE: gather -> RGCN -> SE -> 5x5 convs).

Sharding: 4 documents x 2 cores each. Each pair replicates the cheap upstream
(mention/link/ea gathers -> RGCN -> fmap/SE) and splits the dominant 5x5 conv
stack by output channels, with two intra-pair AllGathers; output halves are
assembled on host. All index-driven gathers happen on host (pure data
movement; one SPMD program serves all 8 cores), all dense math on device.

v2 scheduling notes (driven by the TimelineSim p-state model):
- The PE clock ramps only after ~3us of continuous execution and drops back
  after long (>~3.5us) idles. Warm-up matmuls on a memset tile start the run
  at t~0 so real matmuls execute at full rate; keepalive 1-row matmuls pinned
  to mid-stall DMA completions keep every idle window under the reset
  threshold.
- All large loads stream on the Pool(gpsimd) DMA queue in first-use order,
  split into pieces so consumers wait per-piece.
- type-embedding columns of RGCN layer 0 are folded on host into a constant
  bias matrix B0 = sum_r A_r T Wrel0[512:] + T Wself0[512:] (pure weight
  preprocessing), making all 4 layers uniform 512-contraction.
- SE weights, attl and the fmap/SE intermediates are bf16 (less DMA, same
  matmul rate); the x/W_trans path stays f32r.
- conv relu outputs are stored compact for the pair exchange (fast DMA) and
  padded via a parallel on-chip copy; in solo mode the gather round-trip is
  emulated with 2 hops instead of 3.
"""

import numpy as np
import ml_dtypes

import concourse.bacc as bacc
import concourse.tile as tile
from concourse import mybir
from concourse.bass_utils import run_bass_kernel_spmd

F32 = mybir.dt.float32
F32R = mybir.dt.float32r
BF16 = mybir.dt.bfloat16
FP8 = mybir.dt.float8e4
AF = mybir.ActivationFunctionType
ALU = mybir.AluOpType

NB, H, C, HID, EMB = 4, 12, 1024, 768, 512
E, M, L, SPAN = 22, 4, 16, 32
TD, INTER = 20, 256
NN = E + E * M + L
NREL, NLAYERS = 3, 4
EM, EMH, HS, LS = E * M, E * M * H, H * SPAN, L * SPAN
EE = E * E              # 484
PADW = 26 * 26          # 676 padded 26x26 image
N_CORES = 8
NWARM = 11


def _build_adj():
    A = np.zeros((NREL, NN, NN), np.float32)
    for e in range(E):
        for m in range(M):
            mi = E + e * M + m
            A[0, e, mi] = A[0, mi, e] = 1.0
            for m2 in range(M):
                if m2 != m:
                    A[1, mi, E + e * M + m2] = 1.0
            li = E + E * M + ((e * M + m) % L)
            A[2, mi, li] = A[2, li, mi] = 1.0
    A = A / (A.sum(-1, keepdims=True) + 1e-5)
    return A


_TYPES = np.concatenate([np.zeros(E, np.int32), np.ones(EM, np.int32),
                         np.full(L, 2, np.int32)])


def _const_layout():
    lay = {}
    c = 0

    def add(nm, cols):
        nonlocal c
        lay[nm] = (c, cols)
        c += cols
    add("onescol", 1)
    add("g2T", E)
    for kc in range(4):
        add(f"sumT{kc}", NN)
    return lay, c


def _ctrb_layout():
    lay = {}
    c = 0

    def add(nm, cols):
        nonlocal c
        lay[nm] = (c, cols)
        c += cols
    for kc in range(6):
        add(f"wtr{kc}", EMB)
    add("brow", EMB)
    add("onesrow", 128)
    add("onespad", 110)
    return lay, c


def _constf_layout():
    lay = {}
    c = 0

    def add(nm, cols):
        nonlocal c
        lay[nm] = (c, cols)
        c += cols
    for nm, nch in (("ses1", 2), ("seb1", 2), ("fcs1", 2), ("fcb1", 2),
                    ("ses2", 4), ("seb2", 4), ("fcs2", 4), ("fcb2", 4)):
        for kc in range(nch):
            add(f"{nm}{kc}", 1)
    add("b1h", 1)
    add("b2h", 1)
    add("b3h0", 1)
    add("b3h1", 1)
    add("mtop", 1)
    add("mbot", 1)
    add("identf", 128)
    return lay, c


def _actr_layout():
    lay = {}
    c = 0

    def add(nm, cols):
        nonlocal c
        lay[nm] = (c, cols)
        c += cols
    for kc in range(6):
        add(f"xmT{kc}", 110)     # zero-padded: mention cols at 22..110
    for kc in range(6):
        add(f"xspT{kc}", LS)
    return lay, c


def _sew_layout():
    lay = {}
    c = 0

    def add(nm, cols):
        nonlocal c
        lay[nm] = (c, cols)
        c += cols
    for kc in range(4):
        add(f"fsw1T{kc}", INTER)
    for kc in range(4):
        add(f"fcw1T{kc}", INTER)
    for kc in range(2):
        add(f"fsw2T{kc}", EMB)
    for kc in range(2):
        add(f"fcw2T{kc}", EMB)
    return lay, c


def _attb_layout():
    lay = {}
    c = 0

    def add(nm, cols):
        nonlocal c
        lay[nm] = (c, cols)
        c += cols
    for kc in range(3):
        add(f"attl{kc}", LS)
    add("onesb", 1)
    return lay, c


_LAY_R, _CR = _const_layout()
_LAY_T, _CT = _ctrb_layout()
_LAY_F, _CF = _constf_layout()
_LAY_A, _CA = _actr_layout()
_LAY_S, _CS = _sew_layout()
_LAY_B, _CB = _attb_layout()


def build_program(solo=False, stages=4, dbg=None):
    nc = bacc.Bacc("TRN2", target_bir_lowering=False, debug=False)

    def din(name, shape, dt=F32R):
        return nc.dram_tensor(name, list(shape), dt, kind="ExternalInput").ap()

    constr_d = din("constr", [128, _CR])
    ctrb_d = din("ctrb", [128, _CT], BF16)
    constf_d = din("constf", [128, _CF], F32)
    actr_d = din("actr", [128, _CA], BF16)
    attb_d = din("attb", [128, _CB], BF16)
    sewb_d = din("sewb", [128, _CS], BF16)
    xp_d = din("xp", [128, 8 * HID], BF16)
    amp_d = din("amp", [128, 9 * C], BF16)
    gTb_d = din("gTb", [128, 9 * E], BF16)
    wstp_d = [din(f"wstp{i}", [128, 16 * EMB], BF16) for i in range(4)]
    b0_d = din("b0b", [NN, EMB], BF16)
    w1sb_d = din("w1sb", [4, 128, 25 * 128], BF16)
    w2sb_d = din("w2sb", [2, 128, 25 * 128], BF16)
    w3sb_d = din("w3sb", [2, 128, 25 * 256], BF16)
    aallTb_d = din("aallTb", [NN, (NREL + 1) * NN], BF16)
    identb_d = din("identb", [128, 128], BF16)

    out_d = nc.dram_tensor("out", [256, EE], F32, kind="ExternalOutput").ap()
    dbg_d = nc.dram_tensor("dbg", [128, 1024], F32,
                           kind="ExternalOutput").ap() if dbg else None

    groups = [[0, 1], [2, 3], [4, 5], [6, 7]]

    with tile.TileContext(nc) as tc:
      with tc.tile_pool(name="pconst", bufs=1) as pconst, \
           tc.tile_pool(name="pwork", bufs=1) as pwork, \
           tc.tile_pool(name="pdram", bufs=1, space="DRAM") as pdram:

        constr = pconst.tile([128, _CR], F32R)
        ctrb = pconst.tile([128, _CT], BF16)
        constf = pconst.tile([128, _CF], F32)
        sewb = pconst.tile([128, _CS], BF16)
        identb = pconst.tile([128, 128], BF16)
        aallTb = pconst.tile([NN, (NREL + 1) * NN], BF16)
        b0sb = pconst.tile([NN, EMB], BF16)
        w1 = [pconst.tile([128, 25 * 128], BF16, tag=f"w1_{kc}",
                          name=f"w1_{kc}") for kc in range(4)]

        def cr(nm, rows=128):
            c0, cols = _LAY_R[nm]
            return constr[0:rows, c0:c0 + cols]

        def cf(nm, rows=128):
            c0, cols = _LAY_F[nm]
            return constf[0:rows, c0:c0 + cols]

        def cs(nm, rows=128):
            c0, cols = _LAY_S[nm]
            return sewb[0:rows, c0:c0 + cols]

        def ct(nm, rows=128):
            c0, cols = _LAY_T[nm]
            return ctrb[0:rows, c0:c0 + cols]

        wtr = [ct(f"wtr{kc}") for kc in range(6)]
        brow = ct("brow", rows=1)
        onesrow = ct("onesrow", rows=1)
        onespad = ct("onespad", rows=1)
        g2T = cr("g2T", rows=110)
        sumT = [cr(f"sumT{kc}") for kc in range(4)]
        sew = {nm: [cs(f"{nm}{kc}") for kc in range(n)]
               for nm, n in (("fsw1T", 4), ("fcw1T", 4), ("fsw2T", 2),
                             ("fcw2T", 2))}
        sev = {nm: [cf(f"{nm}{kc}") for kc in range(n)]
               for nm, n in (("ses1", 2), ("seb1", 2), ("fcs1", 2), ("fcb1", 2),
                             ("ses2", 4), ("seb2", 4), ("fcs2", 4),
                             ("fcb2", 4))}
        b1h = cf("b1h")
        b2h = cf("b2h")
        b3h = [cf("b3h0"), cf("b3h1")]
        ident = cf("identf")

        # persistent intermediates
        h0b = pwork.tile([NN, EMB], BF16)
        ectxT_sb = [pwork.tile([128, E], F32, tag=f"ectxT{i}", name=f"ectxT{i}")
                    for i in range(4)]
        fusedp = [pwork.tile([128, PADW], BF16, tag=f"fusedp{i}",
                             name=f"fusedp{i}") for i in range(4)]
        own1 = pwork.tile([128, PADW], BF16)
        oth1 = pwork.tile([128, PADW], BF16)
        own2 = pwork.tile([128, PADW], BF16)
        oth2 = pwork.tile([128, PADW], BF16)

        with tc.tile_pool(name="prgw", bufs=1) as prgw, \
             tc.tile_pool(name="pwarm", bufs=1, space="PSUM") as pwarm:
          # warm-up: keep the PE p-state ramp alive from t~0; the psum bank
          # stays reserved through stage 2 so no WAR sem delays stage 1
          wu = pconst.tile([128, 512], BF16)
          nc.vector.memset(wu[:], 0.0)
          wup = pwarm.tile([128, 256], F32)
          for i in range(NWARM):
              nc.tensor.matmul(wup[:], wu[:, 0:128], wu[:, 0:256],
                               start=True, stop=True)
          # touch every activation function once while ACT is idle so the
          # act-table loads (1283ns each) happen here, not mid-pipeline
          wua = pconst.tile([1, 8], F32)
          for fn_ in (AF.Copy, AF.Exp, AF.Ln, AF.Relu, AF.Sigmoid,
                      AF.Square, AF.Identity):
              nc.scalar.activation(wua[:, 0:1], wu[0:1, 0:1], fn_)
          wstp_t = [prgw.tile([128, 16 * EMB], BF16, tag=f"wstp{l}",
                              name=f"wstp{l}") for l in range(4)]

          with tc.tile_pool(name="pbig", bufs=1) as pbig:
            gTb = pbig.tile([128, 9 * E], BF16)
            xp = pbig.tile([128, 8 * HID], BF16)
            ampt = [pbig.tile([128, 3 * C], BF16, tag=f"amp{g}", name=f"amp{g}")
                    for g in range(3)]
            actr = pbig.tile([128, _CA], BF16)
            attb = pbig.tile([128, _CB], BF16)

            # ------- stage-1 stream on the Pool(gpsimd) DMA queue, in
            # PE-consumption order; bulk weights go on the HWDGE(sync) queue
            # whose dispatch is cheaper, so neither queue's dispatch paces
            # its transfers.
            for g in range(2):
                nc.gpsimd.dma_start(ampt[g][:],
                                    amp_d[:, g * 3 * C:(g + 1) * 3 * C])
            # chunk 8 has only 32 valid rows (EMH=1056): skip the zero rows
            nc.gpsimd.dma_start(ampt[2][:, 0:2 * C], amp_d[:, 6 * C:8 * C])
            nc.gpsimd.dma_start(ampt[2][0:32, 2 * C:3 * C],
                                amp_d[0:32, 8 * C:9 * C])
            for p in range(2):
                nc.gpsimd.dma_start(xp[:, p * 4 * HID:(p + 1) * 4 * HID],
                                    xp_d[:, p * 4 * HID:(p + 1) * 4 * HID])
            nc.gpsimd.dma_start(ctrb[:, 0:6 * EMB], ctrb_d[:, 0:6 * EMB])
            nc.gpsimd.dma_start(ctrb[0:1, 6 * EMB:_CT],
                                ctrb_d[0:1, 6 * EMB:_CT])
            nc.gpsimd.dma_start(constr[:], constr_d[:])
            xm_cols = _LAY_A["xspT0"][0]
            nc.gpsimd.dma_start(actr[:, 0:xm_cols], actr_d[:, 0:xm_cols])
            half_a = xm_cols + 3 * LS
            nc.gpsimd.dma_start(actr[:, xm_cols:half_a],
                                actr_d[:, xm_cols:half_a])
            nc.gpsimd.dma_start(actr[:, half_a:_CA], actr_d[:, half_a:_CA])
            nc.gpsimd.dma_start(attb[:], attb_d[:])

            nc.sync.dma_start(gTb[:], gTb_d[:])
            nc.sync.dma_start(constf[:], constf_d[:])
            nc.sync.dma_start(identb[:], identb_d[:])
            nc.sync.dma_start(aallTb[:], aallTb_d[:])
            # bulk weight stream follows the stage-1 pieces on the same
            # (gpsimd) queue: FIFO ordering gives stage 1 exclusive DMA
            # bandwidth while it is the critical path
            for l in range(4):
                for r in range(4):
                    nc.gpsimd.dma_start(
                        wstp_t[l][:, r * 4 * EMB:(r + 1) * 4 * EMB],
                        wstp_d[l][:, r * 4 * EMB:(r + 1) * 4 * EMB])
            nc.gpsimd.dma_start(b0sb[:], b0_d[:])
            nc.gpsimd.dma_start(sewb[:], sewb_d[:])
            for kc in range(4):
                nc.gpsimd.dma_start(w1[kc][:], w1sb_d[kc])

            def ca(nm, rows=128):
                c0, cols = _LAY_A[nm]
                return actr[0:rows, c0:c0 + cols]

            def cbv(nm, rows=128):
                c0, cols = _LAY_B[nm]
                return attb[0:rows, c0:c0 + cols]

            xmT = [ca(f"xmT{kc}") for kc in range(6)]
            xspT = [ca(f"xspT{kc}") for kc in range(6)]
            attl = [cbv(f"attl{kc}") for kc in range(3)]
            onesb = cbv("onesb")

            # ============ stage 1: gathered-row transforms ============
            expm = pbig.tile([110, EMB], F32R)
            wsb = [pbig.tile([128, 1], F32, tag=f"wsb{i}", name=f"wsb{i}")
                   for i in range(4)]
            wsp = [pbig.tile([128, EMB], F32R, tag=f"wsp{i}", name=f"wsp{i}")
                   for i in range(4)]
            ea_sb = pbig.tile([E, C], F32R)
            eaT = [pbig.tile([128, E], BF16, tag=f"eaT{i}", name=f"eaT{i}")
                   for i in range(8)]
            z_sb = [pbig.tile([128, E], BF16, tag=f"z{i}", name=f"z{i}")
                    for i in range(6)]
            easumT = pbig.tile([1, E], BF16)

            with tc.tile_pool(name="ps1b", bufs=1, space="PSUM") as ps1b:
                # raw (unnormalized) ea; the 1/rowsum normalization is folded
                # into the zt psum->sbuf copy so the transposes don't wait on
                # the reduce chain
                ea_p0 = ps1b.tile([E, 512], F32, tag="ea0", name="ea0")
                ea_p1 = ps1b.tile([E, 512], F32, tag="ea1", name="ea1")
                for kc in range(9):
                    rows = 128 if kc < 8 else 32
                    at = ampt[kc // 3][0:rows, (kc % 3) * C:(kc % 3) * C + C]
                    gt = gTb[0:rows, kc * E:(kc + 1) * E]
                    nc.tensor.matmul(ea_p0[:], gt, at[:, 0:512],
                                     start=(kc == 0), stop=(kc == 8))
                    nc.tensor.matmul(ea_p1[:], gt, at[:, 512:1024],
                                     start=(kc == 0), stop=(kc == 8))
                nc.scalar.activation(ea_sb[:, 0:512], ea_p0[:], AF.Copy)
                nc.scalar.activation(ea_sb[:, 512:1024], ea_p1[:], AF.Copy)
                r0 = pbig.tile([E, 1], F32)
                r1 = pbig.tile([E, 1], F32)
                nc.vector.tensor_reduce(r0[:], ea_p0[:], mybir.AxisListType.X,
                                        ALU.add)
                nc.vector.tensor_reduce(r1[:], ea_p1[:], mybir.AxisListType.X,
                                        ALU.add)
                rsum = pbig.tile([E, 1], F32)
                nc.vector.tensor_tensor(out=rsum[:], in0=r0[:], in1=r1[:],
                                        op=ALU.add)
                rsum2 = pbig.tile([E, 1], F32)
                nc.vector.tensor_scalar(out=rsum2[:], in0=rsum[:],
                                        scalar1=1e-5, scalar2=None,
                                        op0=ALU.add)
                rinv = pbig.tile([E, 1], F32)
                nc.vector.reciprocal(rinv[:], rsum2[:])
                easum = pbig.tile([E, 1], F32)
                nc.vector.tensor_tensor(out=easum[:], in0=rsum[:], in1=rinv[:],
                                        op=ALU.mult)
                for kc in range(8):
                    tp = ps1b.tile([128, E], F32, tag="eaTt", name="eaTt",
                                   bufs=2)
                    nc.tensor.transpose(tp[:],
                                        ea_sb[:, kc * 128:(kc + 1) * 128]
                                        .bitcast(F32), ident[0:E, 0:E])
                    if kc % 2 == 0:
                        nc.scalar.copy(eaT[kc][:], tp[:])
                    else:
                        nc.vector.tensor_copy(out=eaT[kc][:], in_=tp[:])
                tp = ps1b.tile([1, E], F32, tag="easumt", name="easumt")
                nc.tensor.transpose(tp[:], easum[:], ident[0:E, 0:E])
                nc.scalar.copy(easumT[:], tp[:])

            with tc.tile_pool(name="ps1c", bufs=1, space="PSUM") as ps1c:
                zt_ps = [ps1c.tile([E, 384], F32, tag=f"zt_p{i}",
                                   name=f"zt_p{i}") for i in range(2)]
                for kc in range(8):
                    xt = xp[:, kc * HID:(kc + 1) * HID]
                    for hh in range(2):
                        nc.tensor.matmul(zt_ps[hh][:], eaT[kc][:],
                                         xt[:, hh * 384:(hh + 1) * 384],
                                         start=(kc == 0), stop=(kc == 7))
                zt_sb = pbig.tile([E, HID], F32)
                nc.scalar.activation(zt_sb[:, 0:384], zt_ps[0][:], AF.Copy,
                                     scale=rinv[:])
                nc.scalar.activation(zt_sb[:, 384:768], zt_ps[1][:], AF.Copy,
                                     scale=rinv[:])
                for kc in range(6):
                    ztp = ps1c.tile([128, E], F32, tag="ztp", name="ztp",
                                    bufs=2)
                    nc.tensor.transpose(ztp[:],
                                        zt_sb[:, kc * 128:(kc + 1) * 128],
                                        ident[0:E, 0:E])
                    if kc % 2 == 0:
                        nc.scalar.copy(z_sb[kc][:], ztp[:])
                    else:
                        nc.vector.tensor_copy(out=z_sb[kc][:], in_=ztp[:])
                ec2_p = ps1c.tile([E, EMB], F32, tag="ec2", name="ec2")
                for kc in range(6):
                    nc.tensor.matmul(ec2_p[:], z_sb[kc][:], wtr[kc][:],
                                     start=(kc == 0), stop=False)
                nc.tensor.matmul(ec2_p[:], easumT[:], brow[:],
                                 start=False, stop=True)
                ec2_sb = pbig.tile([E, EMB], F32)
                nc.scalar.copy(ec2_sb[:], ec2_p[:])
                for mc in range(4):
                    ecp = ps1c.tile([128, E], F32, tag="ecp", name="ecp",
                                    bufs=2)
                    nc.tensor.transpose(ecp[:],
                                        ec2_sb[:, mc * 128:(mc + 1) * 128],
                                        ident[0:E, 0:E])
                    if mc % 2 == 0:
                        nc.scalar.copy(ectxT_sb[mc][:], ecp[:])
                    else:
                        nc.vector.tensor_copy(out=ectxT_sb[mc][:], in_=ecp[:])

            with tc.tile_pool(name="ps1a", bufs=1, space="PSUM") as ps1a:
                # three psum banks assemble h0's node rows (no DMAs):
                # mentions at rows 22:110 of mrep_p via zero-padded lhsT
                # (stopped early so expm/ep overlap the span work), links at
                # rows 110:126 of h0p via padded lhsT, entities in ep_p.
                h0p = ps1a.tile([128, EMB], F32, tag="h0p", name="h0p")
                mrep_p = ps1a.tile([128, EMB], F32, tag="mrep_p", name="mrep_p")
                ep_p = ps1a.tile([E, EMB], F32, tag="ep_p", name="ep_p")
                for kc in range(6):
                    nc.tensor.matmul(mrep_p[0:110, :], xmT[kc][:, 0:110],
                                     wtr[kc][:], start=(kc == 0), stop=False)
                nc.tensor.matmul(mrep_p[0:110, :], onespad[:], brow[:],
                                 start=False, stop=True)
                nc.scalar.activation(expm[:], mrep_p[0:110, :], AF.Exp)
                nc.tensor.matmul(ep_p[:], g2T[:], expm[:], start=True,
                                 stop=True)


                # w = colsum(attl) / 384 first, so the span transform can be
                # scaled directly on the psum->sbuf copy (no staging tiles)
                for mc in range(4):
                    w_p = ps1a.tile([128, 1], F32, tag="w_p", name="w_p",
                                    bufs=1)
                    for kc in range(3):
                        nc.tensor.matmul(w_p[:],
                                         attl[kc][:, mc * 128:(mc + 1) * 128],
                                         onesb[:],
                                         start=(kc == 0), stop=(kc == 2))
                    nc.scalar.activation(wsb[mc][:], w_p[:], AF.Copy,
                                         scale=1.0 / (H * SPAN))
                for mc in range(4):
                    sp_p = ps1a.tile([128, EMB], F32, tag="sp_p", name="sp_p",
                                     bufs=2)
                    for kc in range(6):
                        nc.tensor.matmul(sp_p[:],
                                         xspT[kc][:, mc * 128:(mc + 1) * 128],
                                         wtr[kc][:], start=(kc == 0),
                                         stop=False)
                    nc.tensor.matmul(sp_p[:], onesrow[:], brow[:],
                                     start=False, stop=True)
                    nc.scalar.activation(wsp[mc][:], sp_p[:], AF.Copy,
                                         scale=wsb[mc][:])
                if dbg == 'link':
                    dbgt = pwork.tile([128, 1024], F32, tag="dbgt",
                                      name="dbgt")
                    for mc in range(4):
                        nc.vector.tensor_copy(out=dbgt[:, mc:mc + 1],
                                              in_=wsb[mc][:])
                    nc.vector.tensor_copy(out=dbgt[:, 8:8 + EMB],
                                          in_=wsp[0][:])
                    nc.sync.dma_start(dbg_d[:], dbgt[:])
                # links -> rows 110:126 via padded lhsT, own group
                for kc in range(4):
                    nc.tensor.matmul(h0p[0:NN, :], sumT[kc][:], wsp[kc][:],
                                     start=(kc == 0), stop=(kc == 3))
                # assemble h0b: links(+junk), then mentions, then entities
                nc.scalar.copy(h0b[0:NN, :], h0p[0:NN, :])
                nc.scalar.copy(h0b[0:110, :], mrep_p[0:110, :])
                nc.scalar.activation(h0b[0:E, :], ep_p[:], AF.Ln)
                if dbg == 'h0b':
                    dbgt = pwork.tile([128, 1024], F32, tag="dbgt",
                                      name="dbgt")
                    nc.vector.tensor_copy(out=dbgt[0:NN, 0:EMB], in_=h0b[:])
                    nc.sync.dma_start(dbg_d[:], dbgt[:])
                if dbg == 'ea':
                    dbgt = pwork.tile([128, 1024], F32, tag="dbgt",
                                      name="dbgt")
                    nc.vector.tensor_copy(out=dbgt[0:E, 0:C], in_=ea_sb[:])
                    nc.vector.tensor_copy(out=dbgt[32:32 + E, 0:1],
                                          in_=rinv[:])
                    nc.vector.tensor_copy(out=dbgt[64:64 + E, 0:HID],
                                          in_=zt_sb[:])
                    nc.sync.dma_start(dbg_d[:], dbgt[:])

          # pbig closed: stage-1 inputs freed
          # zero the conv pad buffers on the (now mostly idle) DVE
          for t_ in fusedp + [own1, oth1, own2, oth2]:
              nc.vector.memset(t_[:], 0.0)

          if stages >= 2:
            # ============ stage 2: RGCN (4 uniform layers) ============
            ecT = [pwork.tile([128, E], F32R, tag=f"ecT{i}", name=f"ecT{i}")
                   for i in range(4)]
            with tc.tile_pool(name="prg", bufs=2) as prg, \
                 tc.tile_pool(name="psr", bufs=1, space="PSUM") as psr:
                h = h0b
                for layer in range(NLAYERS):
                    wstp = wstp_t[layer]
                    u_sb = []
                    for si in range(4):
                        u_p = psr.tile([128, (NREL + 1) * NN], F32, tag="u_p",
                                       name="u_p", bufs=2)
                        nc.tensor.matmul(u_p[:],
                                         h[0:NN, si * 128:(si + 1) * 128],
                                         aallTb[:], start=True, stop=True)
                        u = prg.tile([128, (NREL + 1) * NN], BF16,
                                     tag=f"u{si}", name=f"u{si}")
                        if si % 2 == 0:
                            nc.scalar.copy(u[:], u_p[:])
                        else:
                            nc.vector.tensor_copy(out=u[:], in_=u_p[:])
                        u_sb.append(u)
                    y_p = psr.tile([NN, EMB], F32, tag="y_p", name="y_p")
                    k_mm = 0
                    for r in range(NREL + 1):
                        for si in range(4):
                            nc.tensor.matmul(
                                y_p[:], u_sb[si][:, r * NN:(r + 1) * NN],
                                wstp[:, (r * 4 + si) * EMB:
                                     (r * 4 + si + 1) * EMB],
                                start=(k_mm == 0),
                                stop=(k_mm == 15 and layer > 0))
                            k_mm += 1
                    if layer == 0:
                        nc.tensor.matmul(y_p[:], identb[0:NN, 0:NN], b0sb[:],
                                         start=False, stop=True)
                    hn = prg.tile([NN, EMB], BF16, tag="h_next", name="h_next")
                    nc.scalar.activation(hn[:, 0:256], y_p[:, 0:256], AF.Relu)
                    nc.scalar.activation(hn[:, 256:512], y_p[:, 256:512],
                                         AF.Relu)
                    h = hn
                if dbg == 'h4':
                    dbgt = pwork.tile([128, 1024], F32, tag="dbgt",
                                      name="dbgt")
                    nc.vector.tensor_copy(out=dbgt[0:NN, 0:EMB], in_=h[:])
                    nc.sync.dma_start(dbg_d[:], dbgt[:])

                for mc in range(4):
                    tp = psr.tile([128, E], F32, tag="est", name="est", bufs=2)
                    nc.tensor.matmul(tp[:], h[0:E, mc * 128:(mc + 1) * 128],
                                     identb[0:E, 0:E], start=True, stop=True)
                    nc.vector.tensor_tensor(out=ecT[mc][:], in0=tp[:],
                                            in1=ectxT_sb[mc][:], op=ALU.add)

        if dbg == 'ecT':
            dbgt2 = pwork.tile([128, 1024], F32, tag="dbgt", name="dbgt")
            for mc in range(4):
                nc.vector.tensor_copy(out=dbgt2[:, mc * 32:mc * 32 + E],
                                      in_=ecT[mc][:])
            nc.sync.dma_start(dbg_d[:], dbgt2[:])

        if stages >= 3:
          # ============ stage 3: fmap + SE (bf16) ============
          fmap = [pwork.tile([128, EE], BF16, tag=f"fmap{i}", name=f"fmap{i}")
                  for i in range(4)]
          pooled = [pwork.tile([128, 1], BF16, tag=f"pool{i}", name=f"pool{i}")
                    for i in range(4)]
          for mc in range(4):
              o6v = fmap[mc][:].rearrange("p (i j) -> p i j", i=E)
              in0 = ecT[mc][:].rearrange("p (i j) -> p i j", j=1) \
                  .to_broadcast([128, E, E])
              in1 = ecT[mc][:].rearrange("p (o j) -> p o j", o=1) \
                  .to_broadcast([128, E, E])
              nc.vector.tensor_tensor(out=o6v, in0=in0, in1=in1, op=ALU.mult)
              rs = pwork.tile([128, 1], F32, tag=f"rs{mc}", name=f"rs{mc}")
              nc.vector.tensor_reduce(rs[:], ecT[mc][:], mybir.AxisListType.X,
                                      ALU.add)
              nc.scalar.activation(pooled[mc][:], rs[:], AF.Square,
                                   scale=1.0 / E)

          with tc.tile_pool(name="pse", bufs=1) as pse, \
               tc.tile_pool(name="psse", bufs=1, space="PSUM") as psse:
              c1_sb = [pse.tile([128, 1], BF16, tag=f"c1_{i}", name=f"c1_{i}")
                       for i in range(2)]
              for oc in range(2):
                  c1_p = psse.tile([128, 1], F32, tag="cse", name="c1p")
                  for mc in range(4):
                      nc.tensor.matmul(
                          c1_p[:], sew["fcw1T"][mc][:, oc * 128:(oc + 1) * 128],
                          pooled[mc][:], start=(mc == 0), stop=(mc == 3))
                  c1t = pse.tile([128, 1], F32, tag="c1t", name="c1t",
                                 bufs=2)
                  nc.vector.scalar_tensor_tensor(out=c1t[:], in0=c1_p[:],
                                                 scalar=sev["fcs1"][oc][:],
                                                 in1=sev["fcb1"][oc][:],
                                                 op0=ALU.mult, op1=ALU.add)
                  nc.vector.tensor_scalar(out=c1_sb[oc][:], in0=c1t[:],
                                          scalar1=0.0, scalar2=None,
                                          op0=ALU.max)
              cbb = [pse.tile([128, 1], F32, tag=f"cbb{i}", name=f"cbb{i}")
                     for i in range(4)]
              for mc in range(4):
                  c2_p = psse.tile([128, 1], F32, tag="cse", name="c2p")
                  for kc in range(2):
                      nc.tensor.matmul(
                          c2_p[:], sew["fcw2T"][kc][:, mc * 128:(mc + 1) * 128],
                          c1_sb[kc][:], start=(kc == 0), stop=(kc == 1))
                  cb = pse.tile([128, 1], F32, tag=f"cb{mc}", name=f"cb{mc}")
                  nc.vector.scalar_tensor_tensor(out=cb[:], in0=c2_p[:],
                                                 scalar=sev["fcs2"][mc][:],
                                                 in1=sev["fcb2"][mc][:],
                                                 op0=ALU.mult, op1=ALU.add)
                  nc.vector.tensor_tensor(out=cbb[mc][:], in0=cb[:],
                                          in1=sev["seb2"][mc][:], op=ALU.add)

              s1_sb = [pse.tile([128, EE], BF16, tag=f"s1_{i}", name=f"s1_{i}")
                       for i in range(2)]
              for oc in range(2):
                  s1_p = psse.tile([128, EE], F32, tag="s1p", name="s1p",
                                   bufs=2)
                  for mc in range(4):
                      nc.tensor.matmul(
                          s1_p[:], sew["fsw1T"][mc][:, oc * 128:(oc + 1) * 128],
                          fmap[mc][:], start=(mc == 0), stop=(mc == 3))
                  nc.scalar.activation(s1_sb[oc][:], s1_p[:], AF.Relu,
                                       bias=sev["seb1"][oc][:],
                                       scale=sev["ses1"][oc][:])
              def s2_chunk(mc):
                  s2_p = psse.tile([128, EE], F32, tag="s2p", name="s2p",
                                   bufs=1)
                  for kc in range(2):
                      nc.tensor.matmul(
                          s2_p[:], sew["fsw2T"][kc][:, mc * 128:(mc + 1) * 128],
                          s1_sb[kc][:], start=(kc == 0), stop=(kc == 1))
                  sig = pse.tile([128, EE], BF16, tag="sig", name="sig",
                                 bufs=2)
                  nc.scalar.activation(sig[:], s2_p[:], AF.Sigmoid,
                                       bias=cbb[mc][:], scale=sev["ses2"][mc][:])
                  outv = fusedp[mc][:].rearrange("p (i j) -> p i j",
                                                 j=26)[:, 2:24, 2:24]
                  nc.vector.tensor_tensor(
                      out=outv,
                      in0=fmap[mc][:].rearrange("p (i j) -> p i j", i=E),
                      in1=sig[:].rearrange("p (i j) -> p i j", i=E),
                      op=ALU.mult)

              def keepalive(dep_ap):
                  ka = psse.tile([1, 1], F32, tag="cse", name="ka")
                  nc.tensor.matmul(ka[:], dep_ap, dep_ap, start=True,
                                   stop=True)

              if stages < 4:
                  for mc in range(4):
                      s2_chunk(mc)

              if stages >= 4:
                # ============ stage 4: conv stack ============
                def tap_view(padt, tap):
                    dy, dx = tap // 5, tap % 5
                    return padt[:].rearrange("p (i j) -> p i j",
                                             j=26)[:, dy:dy + 22, dx:dx + 22]

                with tc.tile_pool(name="pcw", bufs=1) as pcw, \
                     tc.tile_pool(name="psc", bufs=1, space="PSUM") as psc:
                    w2 = []
                    for kc in range(2):
                        t = pcw.tile([128, 25 * 128], BF16, tag=f"w2_{kc}",
                                     name=f"w2_{kc}")
                        nc.gpsimd.dma_start(t[:], w2sb_d[kc])
                        w2.append(t)
                    w3 = []
                    for kc in range(2):
                        t = pcw.tile([128, 25 * 256], BF16, tag=f"w3_{kc}",
                                     name=f"w3_{kc}")
                        nc.gpsimd.dma_start(t[:], w3sb_d[kc])
                        w3.append(t)

                    mtop = cf("mtop")
                    mbot = cf("mbot")

                    def interior(t_):
                        return t_[:].rearrange("p (i j) -> p i j",
                                               j=26)[:, 2:24, 2:24]

                    # conv1, interleaved with s2 chunks; keepalives bridge
                    # any DMA-late window at full p-state
                    s2_chunk(0)
                    s2_chunk(1)
                    keepalive(b0sb[0:1, 0:1])
                    keepalive(sewb[0:1, _CS - 1:_CS])
                    r1_p = psc.tile([128, EE], F32, tag="convp", name="convp",
                                    bufs=1)
                    first = True
                    for kc in range(4):
                        if kc + 2 < 4:
                            s2_chunk(kc + 2)
                        if kc + 1 < 4:
                            keepalive(w1[kc][0:1, 0:1])
                        for tap in range(25):
                            nc.tensor.matmul(r1_p[:],
                                             w1[kc][:, tap * 128:(tap + 1) * 128],
                                             tap_view(fusedp[kc], tap),
                                             start=first,
                                             stop=(kc == 3 and tap == 24))
                            first = False
                    r1c = pcw.tile([128, EE], BF16)
                    nc.scalar.activation(r1c[:], r1_p[:], AF.Relu, bias=b1h[:])
                    nc.vector.tensor_copy(out=interior(own1), in_=r1c[:])
                    if dbg == 'r1':
                        dbgt3 = pwork.tile([128, 1024], F32, tag="dbgt",
                                           name="dbgt")
                        nc.vector.tensor_copy(out=dbgt3[:, 0:EE], in_=r1c[:])
                        nc.sync.dma_start(dbg_d[:], dbgt3[:])
                    if dbg == 'fused':
                        dbgt4 = pwork.tile([128, 1024], F32, tag="dbgt",
                                           name="dbgt")
                        nc.vector.tensor_copy(
                            out=dbgt4[:, 0:EE],
                            in_=fusedp[0][:].rearrange("p (i j) -> p i j",
                                                       j=26)[:, 2:24, 2:24])
                        nc.sync.dma_start(dbg_d[:], dbgt4[:])

                    r1b = pdram.tile([128, EE], BF16)
                    nc.sync.dma_start(r1b[:], r1c[:])
                    g1t = pcw.tile([128, EE], BF16)
                    g1b = pcw.tile([128, EE], BF16)
                    if solo:
                        nc.sync.dma_start(g1b[:], r1b[:])
                        nc.sync.dma_start(g1t[:], r1b[:])
                    else:
                        r1g = pdram.tile([256, EE], BF16)
                        nc.gpsimd.collective_compute(
                            "AllGather", ALU.bypass, replica_groups=groups,
                            ins=[r1b[:].opt()], outs=[r1g[:].opt()])
                        nc.sync.dma_start(g1b[:], r1g[128:256, :])
                        nc.sync.dma_start(g1t[:], r1g[0:128, :])
                    tmp1 = pcw.tile([128, EE], F32)
                    nc.vector.tensor_scalar(out=tmp1[:], in0=g1b[:],
                                            scalar1=mbot[:], scalar2=None,
                                            op0=ALU.mult)
                    nc.vector.scalar_tensor_tensor(out=interior(oth1),
                                                   in0=g1t[:], scalar=mtop[:],
                                                   in1=tmp1[:], op0=ALU.mult,
                                                   op1=ALU.add)

                    # conv2: own half first, then other half
                    r2_p = psc.tile([128, EE], F32, tag="convp", name="convp2",
                                    bufs=1)
                    for tap in range(25):
                        nc.tensor.matmul(r2_p[:],
                                         w2[0][:, tap * 128:(tap + 1) * 128],
                                         tap_view(own1, tap),
                                         start=(tap == 0), stop=False)
                    for tap in range(25):
                        nc.tensor.matmul(r2_p[:],
                                         w2[1][:, tap * 128:(tap + 1) * 128],
                                         tap_view(oth1, tap),
                                         start=False, stop=(tap == 24))
                    r2c = pcw.tile([128, EE], BF16)
                    nc.scalar.activation(r2c[:], r2_p[:], AF.Relu, bias=b2h[:])
                    nc.vector.tensor_copy(out=interior(own2), in_=r2c[:])

                    r2b = pdram.tile([128, EE], BF16)
                    nc.sync.dma_start(r2b[:], r2c[:])
                    g2t = pcw.tile([128, EE], BF16)
                    g2b = pcw.tile([128, EE], BF16)
                    if solo:
                        nc.sync.dma_start(g2b[:], r2b[:])
                        nc.sync.dma_start(g2t[:], r2b[:])
                    else:
                        r2g = pdram.tile([256, EE], BF16)
                        nc.gpsimd.collective_compute(
                            "AllGather", ALU.bypass, replica_groups=groups,
                            ins=[r2b[:].opt()], outs=[r2g[:].opt()])
                        nc.sync.dma_start(g2b[:], r2g[128:256, :])
                        nc.sync.dma_start(g2t[:], r2g[0:128, :])
                    tmp2 = pcw.tile([128, EE], F32)
                    nc.vector.tensor_scalar(out=tmp2[:], in0=g2b[:],
                                            scalar1=mbot[:], scalar2=None,
                                            op0=ALU.mult)
                    nc.vector.scalar_tensor_tensor(out=interior(oth2),
                                                   in0=g2t[:], scalar=mtop[:],
                                                   in1=tmp2[:], op0=ALU.mult,
                                                   op1=ALU.add)

                    # conv3: both chunks' own-half taps first (hides r2
                    # gather); oc1 accumulates in two pixel-column halves so
                    # the final relu+store overlaps its last taps
                    r3_ps = [psc.tile([128, EE], F32, tag="convp3",
                                      name="convp3", bufs=3) for _ in range(3)]
                    HALF = 242

                    def half_view(padt, tap, hh):
                        v = tap_view(padt, tap)
                        return v[:, hh * 11:(hh + 1) * 11, :]

                    for tap in range(25):
                        nc.tensor.matmul(
                            r3_ps[0][:], w3[0][:, tap * 256:tap * 256 + 128],
                            tap_view(own2, tap), start=(tap == 0), stop=False)
                    for hh in range(2):
                        for tap in range(25):
                            nc.tensor.matmul(
                                r3_ps[1 + hh][:, 0:HALF],
                                w3[0][:, tap * 256 + 128:tap * 256 + 256],
                                half_view(own2, tap, hh), start=(tap == 0),
                                stop=False)
                    for tap in range(25):
                        nc.tensor.matmul(
                            r3_ps[0][:], w3[1][:, tap * 256:tap * 256 + 128],
                            tap_view(oth2, tap), start=False, stop=(tap == 24))
                    o_sb0 = pcw.tile([128, EE], F32, tag="osb0", name="osb0")
                    nc.scalar.activation(o_sb0[:], r3_ps[0][:], AF.Relu,
                                         bias=b3h[0][:])
                    nc.sync.dma_start(out_d[0:128, :], o_sb0[:])
                    o_sb1 = pcw.tile([128, EE], F32, tag="osb1", name="osb1")
                    for hh in range(2):
                        for tap in range(25):
                            nc.tensor.matmul(
                                r3_ps[1 + hh][:, 0:HALF],
                                w3[1][:, tap * 256 + 128:tap * 256 + 256],
                                half_view(oth2, tap, hh), start=False,
                                stop=(tap == 24))
                        nc.scalar.activation(
                            o_sb1[:, hh * HALF:(hh + 1) * HALF],
                            r3_ps[1 + hh][:, 0:HALF], AF.Relu,
                            bias=b3h[1][:])
                        nc.sync.dma_start(
                            out_d[128:256, hh * HALF:(hh + 1) * HALF],
                            o_sb1[:, hh * HALF:(hh + 1) * HALF])

    nc.compile()
    return nc


_NC_CACHE = None


def _get_program():
    global _NC_CACHE
    if _NC_CACHE is None:
        _NC_CACHE = build_program()
    return _NC_CACHE


def _prep_shared(w):
    """Packed weights/constants identical on every core."""
    ADJ = _build_adj()
    out = {}
    constr = np.zeros((128, _CR), np.float32)
    ctrb = np.zeros((128, _CT), np.float32)

    def put(nm, arr):
        c0, cols = _LAY_R[nm]
        r, cc = arr.shape
        constr[0:r, c0:c0 + cc] = arr

    def putt(nm, arr):
        c0, cols = _LAY_T[nm]
        r, cc = arr.shape
        ctrb[0:r, c0:c0 + cc] = arr
    wt = w['W_trans']
    for kc in range(6):
        putt(f"wtr{kc}", wt[kc * 128:(kc + 1) * 128])
    putt("brow", w['b_trans'].reshape(1, EMB))
    put("onescol", np.ones((128, 1), np.float32))
    putt("onesrow", np.ones((128, 128), np.float32))
    onespad = np.zeros((1, 110), np.float32)
    onespad[0, E:E + EM] = 1.0
    putt("onespad", onespad)
    g2T = np.zeros((110, E), np.float32)
    for e in range(E):
        g2T[E + e * M:E + (e + 1) * M, e] = 1.0
    put("g2T", g2T)
    sumT = np.kron(np.eye(L, dtype=np.float32), np.ones((SPAN, 1), np.float32))
    for kc in range(4):
        sp = np.zeros((128, NN), np.float32)
        sp[:, E + EM:NN] = sumT[kc * 128:(kc + 1) * 128]
        put(f"sumT{kc}", sp)
    out['constr'] = constr
    out['ctrb'] = ctrb.astype(ml_dtypes.bfloat16)

    gT = np.zeros((EMH, E), np.float32)
    for e in range(E):
        gT[e * M * H:(e + 1) * M * H, e] = 1.0 / (M * H)
    gTb = np.zeros((128, 9 * E), np.float32)
    for kc in range(9):
        r = min(128, EMH - kc * 128)
        gTb[0:r, kc * E:(kc + 1) * E] = gT[kc * 128:kc * 128 + r]
    out['gTb'] = gTb.astype(ml_dtypes.bfloat16)
    out['aallTb'] = np.concatenate(
        [ADJ[r].T for r in range(NREL)] + [np.eye(NN, dtype=np.float32)],
        axis=1).astype(ml_dtypes.bfloat16)
    out['identb'] = np.eye(128, dtype=np.float32).astype(ml_dtypes.bfloat16)

    sewb = np.zeros((128, _CS), np.float32)

    def puts(nm, arr):
        c0, cols = _LAY_S[nm]
        sewb[0:arr.shape[0], c0:c0 + arr.shape[1]] = arr
    for nm, arr, nch in (("fsw1T", w['fs_w1'].T, 4), ("fcw1T", w['fc_w1'].T, 4),
                         ("fsw2T", w['fs_w2'].T, 2), ("fcw2T", w['fc_w2'].T, 2)):
        for kc in range(nch):
            puts(f"{nm}{kc}",
                 np.ascontiguousarray(arr[kc * 128:(kc + 1) * 128]))
    out['sewb'] = sewb.astype(ml_dtypes.bfloat16)

    constf = np.zeros((128, _CF), np.float32)

    def putf(nm, arr):
        c0, cols = _LAY_F[nm]
        constf[0:arr.shape[0], c0:c0 + 1] = arr.reshape(-1, 1)
    vecs = {"ses1": w['fs_g1'], "seb1": w['fs_b1'] * w['fs_g1'] + w['fs_be1'],
            "fcs1": w['fc_g1'], "fcb1": w['fc_b1'] * w['fc_g1'] + w['fc_be1'],
            "ses2": w['fs_g2'], "seb2": w['fs_b2'] * w['fs_g2'] + w['fs_be2'],
            "fcs2": w['fc_g2'], "fcb2": w['fc_b2'] * w['fc_g2'] + w['fc_be2']}
    for nm, v in vecs.items():
        nch = 2 if v.shape[0] == INTER else 4
        for kc in range(nch):
            putf(f"{nm}{kc}", v[kc * 128:(kc + 1) * 128])
    out['constf_base'] = constf

    # RGCN weights, r-major pieces; layer-0 type-embed columns folded into B0
    T = np.ascontiguousarray(w['type_embed'][_TYPES])          # [126, 20]
    B0 = T @ w['rgcn_Wself0'][EMB:EMB + TD]
    for r in range(NREL):
        B0 = B0 + ADJ[r] @ (T @ w['rgcn_Wrel0'][r, EMB:EMB + TD])
    out['b0b'] = np.ascontiguousarray(B0).astype(ml_dtypes.bfloat16)
    for layer in range(NLAYERS):
        if layer == 0:
            mats = [w['rgcn_Wrel0'][r, 0:EMB] for r in range(NREL)] + \
                   [w['rgcn_Wself0'][0:EMB]]
        else:
            mats = [w['rgcn_Wrel'][layer - 1, r] for r in range(NREL)] + \
                   [w['rgcn_Wself'][layer - 1]]
        p = np.zeros((128, 16 * EMB), np.float32)
        for r in range(4):
            for si in range(4):
                p[:, (r * 4 + si) * EMB:(r * 4 + si + 1) * EMB] = \
                    mats[r][si * 128:(si + 1) * 128]
        out[f'wstp{layer}'] = p.astype(ml_dtypes.bfloat16)
    return out


def _prep_conv_half(w, half, constf_base):
    out = {}
    w1 = w['cr_w1'][half * 128:(half + 1) * 128]
    out['w1sb'] = np.ascontiguousarray(
        w1.transpose(1, 2, 3, 0).reshape(4, 128, 25 * 128)).astype(
            ml_dtypes.bfloat16)
    w2 = w['cr_w2'][half * 128:(half + 1) * 128]
    w2p = w2.transpose(1, 2, 3, 0).reshape(2, 128, 25 * 128)
    order = [half, 1 - half]
    out['w2sb'] = np.ascontiguousarray(w2p[order]).astype(ml_dtypes.bfloat16)
    w3 = w['cr_w3'][half * 256:(half + 1) * 256]
    w3p = w3.transpose(1, 2, 3, 0).reshape(2, 128, 25 * 256)
    out['w3sb'] = np.ascontiguousarray(w3p[order]).astype(ml_dtypes.bfloat16)
    constf = constf_base.copy()

    def putf(nm, arr):
        c0, cols = _LAY_F[nm]
        constf[0:arr.shape[0], c0:c0 + 1] = arr.reshape(-1, 1)
    putf("b1h", w['cr_b1'][half * 128:(half + 1) * 128])
    putf("b2h", w['cr_b2'][half * 128:(half + 1) * 128])
    putf("b3h0", w['cr_b3'][half * 256:half * 256 + 128])
    putf("b3h1", w['cr_b3'][half * 256 + 128:half * 256 + 256])
    putf("mtop", np.full(128, float(half), np.float32))
    putf("mbot", np.full(128, float(1 - half), np.float32))
    c0, cols = _LAY_F["identf"]
    constf[:, c0:c0 + 128] = np.eye(128, dtype=np.float32)
    out['constf'] = constf
    return out


def _prep_doc(x, att, mi, ls):
    out = {}
    mif = mi.reshape(EM)
    attm = np.ascontiguousarray(
        att[:, mif, :].transpose(1, 0, 2).reshape(EMH, C))
    amp = np.zeros((128, 9 * C), np.float32)
    for kc in range(9):
        r = min(128, EMH - kc * 128)
        amp[0:r, kc * C:kc * C + C] = attm[kc * 128:kc * 128 + r]
    out['amp'] = amp.astype(ml_dtypes.bfloat16)
    idx = ls[:, None] + np.arange(SPAN)
    idxf = idx.reshape(LS)
    rows = att[:, idxf, :].reshape(H, L, SPAN, C)
    blocks = np.take_along_axis(rows, idx[None, :, None, :], axis=3)
    attl = blocks.transpose(0, 2, 1, 3).reshape(HS, LS)
    xmT = x[mif].T
    xspT = x[idxf].T
    actr = np.zeros((128, _CA), np.float32)

    def put(nm, arr):
        c0, cols = _LAY_A[nm]
        actr[0:arr.shape[0], c0:c0 + arr.shape[1]] = arr
    for kc in range(6):
        xmp = np.zeros((128, 110), np.float32)
        xmp[:, E:E + EM] = xmT[kc * 128:(kc + 1) * 128]
        put(f"xmT{kc}", xmp)
        put(f"xspT{kc}", xspT[kc * 128:(kc + 1) * 128])
    out['actr'] = actr.astype(ml_dtypes.bfloat16)
    attb = np.zeros((128, _CB), np.float32)

    def putb(nm, arr):
        c0, cols = _LAY_B[nm]
        attb[0:arr.shape[0], c0:c0 + arr.shape[1]] = arr
    for kc in range(3):
        putb(f"attl{kc}", attl[kc * 128:(kc + 1) * 128])
    putb("onesb", np.ones((128, 1), np.float32))
    out['attb'] = attb.astype(ml_dtypes.bfloat16)
    xpk = np.zeros((128, 8 * HID), np.float32)
    for kc in range(8):
        xpk[:, kc * HID:(kc + 1) * HID] = x[kc * 128:(kc + 1) * 128]
    out['xp'] = xpk.astype(ml_dtypes.bfloat16)
    return out


def build_in_maps(inputs):
    w = {}
    for k, v in inputs.items():
        a = np.asarray(v)
        w[k] = a if a.dtype in (np.int32, np.int64) else \
            np.asarray(a, np.float32)
    shared = _prep_shared(w)
    constf_base = shared.pop('constf_base')
    halves = [_prep_conv_half(w, h, constf_base) for h in range(2)]
    seq = np.asarray(inputs['sequence_output'], np.float32)
    att = np.asarray(inputs['attention'], np.float32)
    mi = np.asarray(inputs['mention_idx']).astype(np.int64)
    ls = np.asarray(inputs['link_start']).astype(np.int64)
    docs = [_prep_doc(seq[n], att[n], mi[n], ls[n]) for n in range(NB)]
    in_maps = []
    for core in range(N_CORES):
        n, half = core // 2, core % 2
        m = dict(shared)
        m.update(halves[half])
        m.update(docs[n])
        in_maps.append({k: (np.ascontiguousarray(v)
                            if v.dtype == ml_dtypes.bfloat16
                            else np.ascontiguousarray(v, np.float32))
                        for k, v in m.items()})
    return in_maps


def kernel(**inputs):
    nc = _get_program()
    in_maps = build_in_maps(inputs)
    res = run_bass_kernel_spmd(nc, in_maps, list(range(N_CORES)))
    out = np.zeros((NB, EMB, E, E), np.float32)
    for core in range(N_CORES):
        n, half = core // 2, core % 2
        out[n, half * 256:(half + 1) * 256] = \
            res.results[core]["out"].reshape(256, E, E)
    return out


# revision 73
# speedup vs baseline: 1.0072x; 1.0072x over previous
"""Trainium2 Bass kernel for nn_DocREModel (DocRE: gather -> RGCN -> SE -> 5x5 convs).

Sharding: 4 documents x 2 cores each. Each pair replicates the cheap upstream
(mention/link/ea gathers -> RGCN -> fmap/SE) and splits the dominant 5x5 conv
stack by output channels, with two intra-pair AllGathers; output halves are
assembled on host. All index-driven gathers happen on host (pure data
movement; one SPMD program serves all 8 cores), all dense math on device.

v2 scheduling notes (driven by the TimelineSim p-state model):
- The PE clock ramps only after ~3us of continuous execution and drops back
  after long (>~3.5us) idles. Warm-up matmuls on a memset tile start the run
  at t~0 so real matmuls execute at full rate; keepalive 1-row matmuls pinned
  to mid-stall DMA completions keep every idle window under the reset
  threshold.
- All large loads stream on the Pool(gpsimd) DMA queue in first-use order,
  split into pieces so consumers wait per-piece.
- type-embedding columns of RGCN layer 0 are folded on host into a constant
  bias matrix B0 = sum_r A_r T Wrel0[512:] + T Wself0[512:] (pure weight
  preprocessing), making all 4 layers uniform 512-contraction.
- SE weights, attl and the fmap/SE intermediates are bf16 (less DMA, same
  matmul rate); the x/W_trans path stays f32r.
- conv relu outputs are stored compact for the pair exchange (fast DMA) and
  padded via a parallel on-chip copy; in solo mode the gather round-trip is
  emulated with 2 hops instead of 3.
"""

import numpy as np
import ml_dtypes

import concourse.bacc as bacc
import concourse.tile as tile
from concourse import mybir
from concourse.bass_utils import run_bass_kernel_spmd

F32 = mybir.dt.float32
F32R = mybir.dt.float32r
BF16 = mybir.dt.bfloat16
FP8 = mybir.dt.float8e4
AF = mybir.ActivationFunctionType
ALU = mybir.AluOpType

NB, H, C, HID, EMB = 4, 12, 1024, 768, 512
E, M, L, SPAN = 22, 4, 16, 32
TD, INTER = 20, 256
NN = E + E * M + L
NREL, NLAYERS = 3, 4
EM, EMH, HS, LS = E * M, E * M * H, H * SPAN, L * SPAN
EE = E * E              # 484
PADW = 26 * 26          # 676 padded 26x26 image
N_CORES = 8
NWARM = 11


def _build_adj():
    A = np.zeros((NREL, NN, NN), np.float32)
    for e in range(E):
        for m in range(M):
            mi = E + e * M + m
            A[0, e, mi] = A[0, mi, e] = 1.0
            for m2 in range(M):
                if m2 != m:
                    A[1, mi, E + e * M + m2] = 1.0
            li = E + E * M + ((e * M + m) % L)
            A[2, mi, li] = A[2, li, mi] = 1.0
    A = A / (A.sum(-1, keepdims=True) + 1e-5)
    return A


_TYPES = np.concatenate([np.zeros(E, np.int32), np.ones(EM, np.int32),
                         np.full(L, 2, np.int32)])


def _const_layout():
    lay = {}
    c = 0

    def add(nm, cols):
        nonlocal c
        lay[nm] = (c, cols)
        c += cols
    add("onescol", 1)
    add("g2T", E)
    for kc in range(4):
        add(f"sumT{kc}", NN)
    return lay, c


def _ctrb_layout():
    lay = {}
    c = 0

    def add(nm, cols):
        nonlocal c
        lay[nm] = (c, cols)
        c += cols
    for kc in range(6):
        add(f"wtr{kc}", EMB)
    add("brow", EMB)
    add("onesrow", 128)
    add("onespad", 110)
    return lay, c


def _constf_layout():
    lay = {}
    c = 0

    def add(nm, cols):
        nonlocal c
        lay[nm] = (c, cols)
        c += cols
    for nm, nch in (("ses1", 2), ("seb1", 2), ("fcs1", 2), ("fcb1", 2),
                    ("ses2", 4), ("seb2", 4), ("fcs2", 4), ("fcb2", 4)):
        for kc in range(nch):
            add(f"{nm}{kc}", 1)
    add("b1h", 1)
    add("b2h", 1)
    add("b3h0", 1)
    add("b3h1", 1)
    add("mtop", 1)
    add("mbot", 1)
    add("identf", 128)
    return lay, c


def _actr_layout():
    lay = {}
    c = 0

    def add(nm, cols):
        nonlocal c
        lay[nm] = (c, cols)
        c += cols
    for kc in range(6):
        add(f"xmT{kc}", 110)     # zero-padded: mention cols at 22..110
    for kc in range(6):
        add(f"xspT{kc}", LS)
    return lay, c


def _sew_layout():
    lay = {}
    c = 0

    def add(nm, cols):
        nonlocal c
        lay[nm] = (c, cols)
        c += cols
    for kc in range(4):
        add(f"fsw1T{kc}", INTER)
    for kc in range(4):
        add(f"fcw1T{kc}", INTER)
    for kc in range(2):
        add(f"fsw2T{kc}", EMB)
    for kc in range(2):
        add(f"fcw2T{kc}", EMB)
    return lay, c


def _attb_layout():
    lay = {}
    c = 0

    def add(nm, cols):
        nonlocal c
        lay[nm] = (c, cols)
        c += cols
    for kc in range(3):
        add(f"attl{kc}", LS)
    add("onesb", 1)
    return lay, c


_LAY_R, _CR = _const_layout()
_LAY_T, _CT = _ctrb_layout()
_LAY_F, _CF = _constf_layout()
_LAY_A, _CA = _actr_layout()
_LAY_S, _CS = _sew_layout()
_LAY_B, _CB = _attb_layout()


def build_program(solo=False, stages=4, dbg=None):
    nc = bacc.Bacc("TRN2", target_bir_lowering=False, debug=False)

    def din(name, shape, dt=F32R):
        return nc.dram_tensor(name, list(shape), dt, kind="ExternalInput").ap()

    constr_d = din("constr", [128, _CR])
    ctrb_d = din("ctrb", [128, _CT], BF16)
    constf_d = din("constf", [128, _CF], F32)
    actr_d = din("actr", [128, _CA], BF16)
    attb_d = din("attb", [128, _CB], BF16)
    sewb_d = din("sewb", [128, _CS], BF16)
    xp_d = din("xp", [128, 8 * HID], BF16)
    amp_d = din("amp", [128, 9 * C], BF16)
    gTb_d = din("gTb", [128, 9 * E], BF16)
    wstp_d = [din(f"wstp{i}", [128, 16 * EMB], BF16) for i in range(4)]
    b0_d = din("b0b", [NN, EMB], BF16)
    w1sb_d = din("w1sb", [4, 128, 25 * 128], BF16)
    w2sb_d = din("w2sb", [2, 128, 25 * 128], BF16)
    w3sb_d = din("w3sb", [2, 128, 25 * 256], BF16)
    aallTb_d = din("aallTb", [NN, (NREL + 1) * NN], BF16)
    identb_d = din("identb", [128, 128], BF16)

    out_d = nc.dram_tensor("out", [256, EE], F32, kind="ExternalOutput").ap()
    dbg_d = nc.dram_tensor("dbg", [128, 1024], F32,
                           kind="ExternalOutput").ap() if dbg else None

    groups = [[0, 1], [2, 3], [4, 5], [6, 7]]

    with tile.TileContext(nc) as tc:
      with tc.tile_pool(name="pconst", bufs=1) as pconst, \
           tc.tile_pool(name="pwork", bufs=1) as pwork, \
           tc.tile_pool(name="pdram", bufs=1, space="DRAM") as pdram:

        constr = pconst.tile([128, _CR], F32R)
        ctrb = pconst.tile([128, _CT], BF16)
        constf = pconst.tile([128, _CF], F32)
        sewb = pconst.tile([128, _CS], BF16)
        identb = pconst.tile([128, 128], BF16)
        aallTb = pconst.tile([NN, (NREL + 1) * NN], BF16)
        b0sb = pconst.tile([NN, EMB], BF16)
        w1 = [pconst.tile([128, 25 * 128], BF16, tag=f"w1_{kc}",
                          name=f"w1_{kc}") for kc in range(4)]

        def cr(nm, rows=128):
            c0, cols = _LAY_R[nm]
            return constr[0:rows, c0:c0 + cols]

        def cf(nm, rows=128):
            c0, cols = _LAY_F[nm]
            return constf[0:rows, c0:c0 + cols]

        def cs(nm, rows=128):
            c0, cols = _LAY_S[nm]
            return sewb[0:rows, c0:c0 + cols]

        def ct(nm, rows=128):
            c0, cols = _LAY_T[nm]
            return ctrb[0:rows, c0:c0 + cols]

        wtr = [ct(f"wtr{kc}") for kc in range(6)]
        brow = ct("brow", rows=1)
        onesrow = ct("onesrow", rows=1)
        onespad = ct("onespad", rows=1)
        g2T = cr("g2T", rows=110)
        sumT = [cr(f"sumT{kc}") for kc in range(4)]
        sew = {nm: [cs(f"{nm}{kc}") for kc in range(n)]
               for nm, n in (("fsw1T", 4), ("fcw1T", 4), ("fsw2T", 2),
                             ("fcw2T", 2))}
        sev = {nm: [cf(f"{nm}{kc}") for kc in range(n)]
               for nm, n in (("ses1", 2), ("seb1", 2), ("fcs1", 2), ("fcb1", 2),
                             ("ses2", 4), ("seb2", 4), ("fcs2", 4),
                             ("fcb2", 4))}
        b1h = cf("b1h")
        b2h = cf("b2h")
        b3h = [cf("b3h0"), cf("b3h1")]
        ident = cf("identf")

        # persistent intermediates
        h0b = pwork.tile([NN, EMB], BF16)
        ectxT_sb = [pwork.tile([128, E], F32, tag=f"ectxT{i}", name=f"ectxT{i}")
                    for i in range(4)]
        fusedp = [pwork.tile([128, PADW], BF16, tag=f"fusedp{i}",
                             name=f"fusedp{i}") for i in range(4)]
        own1 = pwork.tile([128, PADW], BF16)
        oth1 = pwork.tile([128, PADW], BF16)
        own2 = pwork.tile([128, PADW], BF16)
        oth2 = pwork.tile([128, PADW], BF16)

        with tc.tile_pool(name="prgw", bufs=1) as prgw, \
             tc.tile_pool(name="pwarm", bufs=1, space="PSUM") as pwarm:
          # warm-up: keep the PE p-state ramp alive from t~0; the psum bank
          # stays reserved through stage 2 so no WAR sem delays stage 1
          wu = pconst.tile([128, 512], BF16)
          nc.vector.memset(wu[:], 0.0)
          wup = pwarm.tile([128, 256], F32)
          for i in range(NWARM):
              nc.tensor.matmul(wup[:], wu[:, 0:128], wu[:, 0:256],
                               start=True, stop=True)
          # touch every activation function once while ACT is idle so the
          # act-table loads (1283ns each) happen here, not mid-pipeline
          wua = pconst.tile([1, 8], F32)
          for fn_ in (AF.Copy, AF.Exp, AF.Ln, AF.Relu, AF.Sigmoid,
                      AF.Square, AF.Identity):
              nc.scalar.activation(wua[:, 0:1], wu[0:1, 0:1], fn_)
          wstp_t = [prgw.tile([128, 16 * EMB], BF16, tag=f"wstp{l}",
                              name=f"wstp{l}") for l in range(4)]

          with tc.tile_pool(name="pbig", bufs=1) as pbig:
            gTb = pbig.tile([128, 9 * E], BF16)
            xp = pbig.tile([128, 8 * HID], BF16)
            ampt = [pbig.tile([128, 3 * C], BF16, tag=f"amp{g}", name=f"amp{g}")
                    for g in range(3)]
            actr = pbig.tile([128, _CA], BF16)
            attb = pbig.tile([128, _CB], BF16)

            # ------- stage-1 stream on the Pool(gpsimd) DMA queue, in
            # PE-consumption order; bulk weights go on the HWDGE(sync) queue
            # whose dispatch is cheaper, so neither queue's dispatch paces
            # its transfers.
            for g in range(2):
                nc.gpsimd.dma_start(ampt[g][:],
                                    amp_d[:, g * 3 * C:(g + 1) * 3 * C])
            # chunk 8 has only 32 valid rows (EMH=1056): skip the zero rows
            nc.gpsimd.dma_start(ampt[2][:, 0:2 * C], amp_d[:, 6 * C:8 * C])
            nc.gpsimd.dma_start(ampt[2][0:32, 2 * C:3 * C],
                                amp_d[0:32, 8 * C:9 * C])
            for p in range(2):
                nc.gpsimd.dma_start(xp[:, p * 4 * HID:(p + 1) * 4 * HID],
                                    xp_d[:, p * 4 * HID:(p + 1) * 4 * HID])
            nc.gpsimd.dma_start(ctrb[:, 0:6 * EMB], ctrb_d[:, 0:6 * EMB])
            nc.gpsimd.dma_start(ctrb[0:1, 6 * EMB:_CT],
                                ctrb_d[0:1, 6 * EMB:_CT])
            nc.gpsimd.dma_start(constr[:], constr_d[:])
            xm_cols = _LAY_A["xspT0"][0]
            nc.gpsimd.dma_start(actr[:, 0:xm_cols], actr_d[:, 0:xm_cols])
            half_a = xm_cols + 3 * LS
            nc.gpsimd.dma_start(actr[:, xm_cols:half_a],
                                actr_d[:, xm_cols:half_a])
            nc.gpsimd.dma_start(actr[:, half_a:_CA], actr_d[:, half_a:_CA])
            nc.gpsimd.dma_start(attb[:], attb_d[:])

            nc.sync.dma_start(gTb[:], gTb_d[:])
            nc.sync.dma_start(constf[:], constf_d[:])
            nc.sync.dma_start(identb[:], identb_d[:])
            nc.sync.dma_start(aallTb[:], aallTb_d[:])
            # bulk weight stream follows the stage-1 pieces on the same
            # (gpsimd) queue: FIFO ordering gives stage 1 exclusive DMA
            # bandwidth while it is the critical path
            for l in range(4):
                for r in range(4):
                    nc.gpsimd.dma_start(
                        wstp_t[l][:, r * 4 * EMB:(r + 1) * 4 * EMB],
                        wstp_d[l][:, r * 4 * EMB:(r + 1) * 4 * EMB])
            nc.gpsimd.dma_start(b0sb[:], b0_d[:])
            nc.gpsimd.dma_start(sewb[:], sewb_d[:])
            for kc in range(4):
                nc.gpsimd.dma_start(w1[kc][:], w1sb_d[kc])

            def ca(nm, rows=128):
                c0, cols = _LAY_A[nm]
                return actr[0:rows, c0:c0 + cols]

            def cbv(nm, rows=128):
                c0, cols = _LAY_B[nm]
                return attb[0:rows, c0:c0 + cols]

            xmT = [ca(f"xmT{kc}") for kc in range(6)]
            xspT = [ca(f"xspT{kc}") for kc in range(6)]
            attl = [cbv(f"attl{kc}") for kc in range(3)]
            onesb = cbv("onesb")

            # ============ stage 1: gathered-row transforms ============
            expm = pbig.tile([110, EMB], F32R)
            wsb = [pbig.tile([128, 1], F32, tag=f"wsb{i}", name=f"wsb{i}")
                   for i in range(4)]
            wsp = [pbig.tile([128, EMB], F32R, tag=f"wsp{i}", name=f"wsp{i}")
                   for i in range(4)]
            ea_sb = pbig.tile([E, C], F32R)
            eaT = [pbig.tile([128, E], BF16, tag=f"eaT{i}", name=f"eaT{i}")
                   for i in range(8)]
            z_sb = [pbig.tile([128, E], BF16, tag=f"z{i}", name=f"z{i}")
                    for i in range(6)]
            easumT = pbig.tile([1, E], BF16)

            with tc.tile_pool(name="ps1b", bufs=1, space="PSUM") as ps1b:
                # raw (unnormalized) ea; the 1/rowsum normalization is folded
                # into the zt psum->sbuf copy so the transposes don't wait on
                # the reduce chain
                ea_p0 = ps1b.tile([E, 512], F32, tag="ea0", name="ea0")
                ea_p1 = ps1b.tile([E, 512], F32, tag="ea1", name="ea1")
                for kc in range(9):
                    rows = 128 if kc < 8 else 32
                    at = ampt[kc // 3][0:rows, (kc % 3) * C:(kc % 3) * C + C]
                    gt = gTb[0:rows, kc * E:(kc + 1) * E]
                    nc.tensor.matmul(ea_p0[:], gt, at[:, 0:512],
                                     start=(kc == 0), stop=(kc == 8))
                    nc.tensor.matmul(ea_p1[:], gt, at[:, 512:1024],
                                     start=(kc == 0), stop=(kc == 8))
                nc.scalar.activation(ea_sb[:, 0:512], ea_p0[:], AF.Copy)
                nc.scalar.activation(ea_sb[:, 512:1024], ea_p1[:], AF.Copy)
                r0 = pbig.tile([E, 1], F32)
                r1 = pbig.tile([E, 1], F32)
                nc.vector.tensor_reduce(r0[:], ea_p0[:], mybir.AxisListType.X,
                                        ALU.add)
                nc.vector.tensor_reduce(r1[:], ea_p1[:], mybir.AxisListType.X,
                                        ALU.add)
                rsum = pbig.tile([E, 1], F32)
                nc.vector.tensor_tensor(out=rsum[:], in0=r0[:], in1=r1[:],
                                        op=ALU.add)
                rsum2 = pbig.tile([E, 1], F32)
                nc.vector.tensor_scalar(out=rsum2[:], in0=rsum[:],
                                        scalar1=1e-5, scalar2=None,
                                        op0=ALU.add)
                rinv = pbig.tile([E, 1], F32)
                nc.vector.reciprocal(rinv[:], rsum2[:])
                easum = pbig.tile([E, 1], F32)
                nc.vector.tensor_tensor(out=easum[:], in0=rsum[:], in1=rinv[:],
                                        op=ALU.mult)
                for kc in range(8):
                    tp = ps1b.tile([128, E], F32, tag="eaTt", name="eaTt",
                                   bufs=2)
                    nc.tensor.transpose(tp[:],
                                        ea_sb[:, kc * 128:(kc + 1) * 128]
                                        .bitcast(F32), ident[0:E, 0:E])
                    if kc % 2 == 0:
                        nc.scalar.copy(eaT[kc][:], tp[:])
                    else:
                        nc.vector.tensor_copy(out=eaT[kc][:], in_=tp[:])
                tp = ps1b.tile([1, E], F32, tag="easumt", name="easumt")
                nc.tensor.transpose(tp[:], easum[:], ident[0:E, 0:E])
                nc.scalar.copy(easumT[:], tp[:])

            with tc.tile_pool(name="ps1c", bufs=1, space="PSUM") as ps1c:
                zt_ps = [ps1c.tile([E, 384], F32, tag=f"zt_p{i}",
                                   name=f"zt_p{i}") for i in range(2)]
                for kc in range(8):
                    xt = xp[:, kc * HID:(kc + 1) * HID]
                    for hh in range(2):
                        nc.tensor.matmul(zt_ps[hh][:], eaT[kc][:],
                                         xt[:, hh * 384:(hh + 1) * 384],
                                         start=(kc == 0), stop=(kc == 7))
                zt_sb = pbig.tile([E, HID], F32)
                nc.scalar.activation(zt_sb[:, 0:384], zt_ps[0][:], AF.Copy,
                                     scale=rinv[:])
                nc.scalar.activation(zt_sb[:, 384:768], zt_ps[1][:], AF.Copy,
                                     scale=rinv[:])
                for kc in range(6):
                    ztp = ps1c.tile([128, E], F32, tag="ztp", name="ztp",
                                    bufs=2)
                    nc.tensor.transpose(ztp[:],
                                        zt_sb[:, kc * 128:(kc + 1) * 128],
                                        ident[0:E, 0:E])
                    if kc % 2 == 0:
                        nc.scalar.copy(z_sb[kc][:], ztp[:])
                    else:
                        nc.vector.tensor_copy(out=z_sb[kc][:], in_=ztp[:])
                ec2_p = ps1c.tile([E, EMB], F32, tag="ec2", name="ec2")
                for kc in range(6):
                    nc.tensor.matmul(ec2_p[:], z_sb[kc][:], wtr[kc][:],
                                     start=(kc == 0), stop=False)
                nc.tensor.matmul(ec2_p[:], easumT[:], brow[:],
                                 start=False, stop=True)
                ec2_sb = pbig.tile([E, EMB], F32)
                nc.scalar.copy(ec2_sb[:], ec2_p[:])
                for mc in range(4):
                    ecp = ps1c.tile([128, E], F32, tag="ecp", name="ecp",
                                    bufs=2)
                    nc.tensor.transpose(ecp[:],
                                        ec2_sb[:, mc * 128:(mc + 1) * 128],
                                        ident[0:E, 0:E])
                    if mc % 2 == 0:
                        nc.scalar.copy(ectxT_sb[mc][:], ecp[:])
                    else:
                        nc.vector.tensor_copy(out=ectxT_sb[mc][:], in_=ecp[:])

            with tc.tile_pool(name="ps1a", bufs=1, space="PSUM") as ps1a:
                # three psum banks assemble h0's node rows (no DMAs):
                # mentions at rows 22:110 of mrep_p via zero-padded lhsT
                # (stopped early so expm/ep overlap the span work), links at
                # rows 110:126 of h0p via padded lhsT, entities in ep_p.
                h0p = ps1a.tile([128, EMB], F32, tag="h0p", name="h0p")
                mrep_p = ps1a.tile([128, EMB], F32, tag="mrep_p", name="mrep_p")
                ep_p = ps1a.tile([E, EMB], F32, tag="ep_p", name="ep_p")
                for kc in range(6):
                    nc.tensor.matmul(mrep_p[0:110, :], xmT[kc][:, 0:110],
                                     wtr[kc][:], start=(kc == 0), stop=False)
                nc.tensor.matmul(mrep_p[0:110, :], onespad[:], brow[:],
                                 start=False, stop=True)
                nc.scalar.activation(expm[:], mrep_p[0:110, :], AF.Exp)
                nc.tensor.matmul(ep_p[:], g2T[:], expm[:], start=True,
                                 stop=True)


                # w = colsum(attl) / 384 first, so the span transform can be
                # scaled directly on the psum->sbuf copy (no staging tiles)
                for mc in range(4):
                    w_p = ps1a.tile([128, 1], F32, tag="w_p", name="w_p",
                                    bufs=1)
                    for kc in range(3):
                        nc.tensor.matmul(w_p[:],
                                         attl[kc][:, mc * 128:(mc + 1) * 128],
                                         onesb[:],
                                         start=(kc == 0), stop=(kc == 2))
                    nc.scalar.activation(wsb[mc][:], w_p[:], AF.Copy,
                                         scale=1.0 / (H * SPAN))
                for mc in range(4):
                    sp_p = ps1a.tile([128, EMB], F32, tag="sp_p", name="sp_p",
                                     bufs=2)
                    for kc in range(6):
                        nc.tensor.matmul(sp_p[:],
                                         xspT[kc][:, mc * 128:(mc + 1) * 128],
                                         wtr[kc][:], start=(kc == 0),
                                         stop=False)
                    nc.tensor.matmul(sp_p[:], onesrow[:], brow[:],
                                     start=False, stop=True)
                    nc.scalar.activation(wsp[mc][:], sp_p[:], AF.Copy,
                                         scale=wsb[mc][:])
                if dbg == 'link':
                    dbgt = pwork.tile([128, 1024], F32, tag="dbgt",
                                      name="dbgt")
                    for mc in range(4):
                        nc.vector.tensor_copy(out=dbgt[:, mc:mc + 1],
                                              in_=wsb[mc][:])
                    nc.vector.tensor_copy(out=dbgt[:, 8:8 + EMB],
                                          in_=wsp[0][:])
                    nc.sync.dma_start(dbg_d[:], dbgt[:])
                # links -> rows 110:126 via padded lhsT, own group
                for kc in range(4):
                    nc.tensor.matmul(h0p[0:NN, :], sumT[kc][:], wsp[kc][:],
                                     start=(kc == 0), stop=(kc == 3))
                # assemble h0b: links(+junk), then mentions, then entities
                nc.scalar.copy(h0b[0:NN, :], h0p[0:NN, :])
                nc.scalar.copy(h0b[0:110, :], mrep_p[0:110, :])
                nc.scalar.activation(h0b[0:E, :], ep_p[:], AF.Ln)
                if dbg == 'h0b':
                    dbgt = pwork.tile([128, 1024], F32, tag="dbgt",
                                      name="dbgt")
                    nc.vector.tensor_copy(out=dbgt[0:NN, 0:EMB], in_=h0b[:])
                    nc.sync.dma_start(dbg_d[:], dbgt[:])
                if dbg == 'ea':
                    dbgt = pwork.tile([128, 1024], F32, tag="dbgt",
                                      name="dbgt")
                    nc.vector.tensor_copy(out=dbgt[0:E, 0:C], in_=ea_sb[:])
                    nc.vector.tensor_copy(out=dbgt[32:32 + E, 0:1],
                                          in_=rinv[:])
                    nc.vector.tensor_copy(out=dbgt[64:64 + E, 0:HID],
                                          in_=zt_sb[:])
                    nc.sync.dma_start(dbg_d[:], dbgt[:])

          # pbig closed: stage-1 inputs freed
          # zero the conv pad buffers on the (now mostly idle) DVE
          for t_ in fusedp + [own1, oth1, own2, oth2]:
              nc.vector.memset(t_[:], 0.0)

          if stages >= 2:
            # ============ stage 2: RGCN (4 uniform layers) ============
            ecT = [pwork.tile([128, E], F32R, tag=f"ecT{i}", name=f"ecT{i}")
                   for i in range(4)]
            with tc.tile_pool(name="prg", bufs=2) as prg, \
                 tc.tile_pool(name="psr", bufs=1, space="PSUM") as psr:
                h = h0b
                for layer in range(NLAYERS):
                    wstp = wstp_t[layer]
                    u_sb = []
                    for si in range(4):
                        u_p = psr.tile([128, (NREL + 1) * NN], F32, tag="u_p",
                                       name="u_p", bufs=4)
                        nc.tensor.matmul(u_p[:],
                                         h[0:NN, si * 128:(si + 1) * 128],
                                         aallTb[:], start=True, stop=True)
                        u = prg.tile([128, (NREL + 1) * NN], BF16,
                                     tag=f"u{si}", name=f"u{si}")
                        if si % 2 == 0:
                            nc.scalar.copy(u[:], u_p[:])
                        else:
                            nc.vector.tensor_copy(out=u[:], in_=u_p[:])
                        u_sb.append(u)
                    y_p = psr.tile([NN, EMB], F32, tag="y_p", name="y_p")
                    k_mm = 0
                    for r in range(NREL + 1):
                        for si in range(4):
                            nc.tensor.matmul(
                                y_p[:], u_sb[si][:, r * NN:(r + 1) * NN],
                                wstp[:, (r * 4 + si) * EMB:
                                     (r * 4 + si + 1) * EMB],
                                start=(k_mm == 0),
                                stop=(k_mm == 15 and layer > 0))
                            k_mm += 1
                    if layer == 0:
                        nc.tensor.matmul(y_p[:], identb[0:NN, 0:NN], b0sb[:],
                                         start=False, stop=True)
                    hn = prg.tile([NN, EMB], BF16, tag="h_next", name="h_next")
                    nc.scalar.activation(hn[:, 0:256], y_p[:, 0:256], AF.Relu)
                    nc.scalar.activation(hn[:, 256:512], y_p[:, 256:512],
                                         AF.Relu)
                    h = hn
                if dbg == 'h4':
                    dbgt = pwork.tile([128, 1024], F32, tag="dbgt",
                                      name="dbgt")
                    nc.vector.tensor_copy(out=dbgt[0:NN, 0:EMB], in_=h[:])
                    nc.sync.dma_start(dbg_d[:], dbgt[:])

                for mc in range(4):
                    tp = psr.tile([128, E], F32, tag="est", name="est", bufs=2)
                    nc.tensor.matmul(tp[:], h[0:E, mc * 128:(mc + 1) * 128],
                                     identb[0:E, 0:E], start=True, stop=True)
                    nc.vector.tensor_tensor(out=ecT[mc][:], in0=tp[:],
                                            in1=ectxT_sb[mc][:], op=ALU.add)

        if dbg == 'ecT':
            dbgt2 = pwork.tile([128, 1024], F32, tag="dbgt", name="dbgt")
            for mc in range(4):
                nc.vector.tensor_copy(out=dbgt2[:, mc * 32:mc * 32 + E],
                                      in_=ecT[mc][:])
            nc.sync.dma_start(dbg_d[:], dbgt2[:])

        if stages >= 3:
          # ============ stage 3: fmap + SE (bf16) ============
          fmap = [pwork.tile([128, EE], BF16, tag=f"fmap{i}", name=f"fmap{i}")
                  for i in range(4)]
          pooled = [pwork.tile([128, 1], BF16, tag=f"pool{i}", name=f"pool{i}")
                    for i in range(4)]
          for mc in range(4):
              o6v = fmap[mc][:].rearrange("p (i j) -> p i j", i=E)
              in0 = ecT[mc][:].rearrange("p (i j) -> p i j", j=1) \
                  .to_broadcast([128, E, E])
              in1 = ecT[mc][:].rearrange("p (o j) -> p o j", o=1) \
                  .to_broadcast([128, E, E])
              nc.vector.tensor_tensor(out=o6v, in0=in0, in1=in1, op=ALU.mult)
              rs = pwork.tile([128, 1], F32, tag=f"rs{mc}", name=f"rs{mc}")
              nc.vector.tensor_reduce(rs[:], ecT[mc][:], mybir.AxisListType.X,
                                      ALU.add)
              nc.scalar.activation(pooled[mc][:], rs[:], AF.Square,
                                   scale=1.0 / E)

          with tc.tile_pool(name="pse", bufs=1) as pse, \
               tc.tile_pool(name="psse", bufs=1, space="PSUM") as psse:
              c1_sb = [pse.tile([128, 1], BF16, tag=f"c1_{i}", name=f"c1_{i}")
                       for i in range(2)]
              for oc in range(2):
                  c1_p = psse.tile([128, 1], F32, tag="cse", name="c1p")
                  for mc in range(4):
                      nc.tensor.matmul(
                          c1_p[:], sew["fcw1T"][mc][:, oc * 128:(oc + 1) * 128],
                          pooled[mc][:], start=(mc == 0), stop=(mc == 3))
                  c1t = pse.tile([128, 1], F32, tag="c1t", name="c1t",
                                 bufs=2)
                  nc.vector.scalar_tensor_tensor(out=c1t[:], in0=c1_p[:],
                                                 scalar=sev["fcs1"][oc][:],
                                                 in1=sev["fcb1"][oc][:],
                                                 op0=ALU.mult, op1=ALU.add)
                  nc.vector.tensor_scalar(out=c1_sb[oc][:], in0=c1t[:],
                                          scalar1=0.0, scalar2=None,
                                          op0=ALU.max)
              cbb = [pse.tile([128, 1], F32, tag=f"cbb{i}", name=f"cbb{i}")
                     for i in range(4)]
              for mc in range(4):
                  c2_p = psse.tile([128, 1], F32, tag="cse", name="c2p")
                  for kc in range(2):
                      nc.tensor.matmul(
                          c2_p[:], sew["fcw2T"][kc][:, mc * 128:(mc + 1) * 128],
                          c1_sb[kc][:], start=(kc == 0), stop=(kc == 1))
                  cb = pse.tile([128, 1], F32, tag=f"cb{mc}", name=f"cb{mc}")
                  nc.vector.scalar_tensor_tensor(out=cb[:], in0=c2_p[:],
                                                 scalar=sev["fcs2"][mc][:],
                                                 in1=sev["fcb2"][mc][:],
                                                 op0=ALU.mult, op1=ALU.add)
                  nc.vector.tensor_tensor(out=cbb[mc][:], in0=cb[:],
                                          in1=sev["seb2"][mc][:], op=ALU.add)

              s1_sb = [pse.tile([128, EE], BF16, tag=f"s1_{i}", name=f"s1_{i}")
                       for i in range(2)]
              for oc in range(2):
                  s1_p = psse.tile([128, EE], F32, tag="s1p", name="s1p",
                                   bufs=2)
                  for mc in range(4):
                      nc.tensor.matmul(
                          s1_p[:], sew["fsw1T"][mc][:, oc * 128:(oc + 1) * 128],
                          fmap[mc][:], start=(mc == 0), stop=(mc == 3))
                  nc.scalar.activation(s1_sb[oc][:], s1_p[:], AF.Relu,
                                       bias=sev["seb1"][oc][:],
                                       scale=sev["ses1"][oc][:])
              def s2_chunk(mc):
                  s2_p = psse.tile([128, EE], F32, tag="s2p", name="s2p",
                                   bufs=1)
                  for kc in range(2):
                      nc.tensor.matmul(
                          s2_p[:], sew["fsw2T"][kc][:, mc * 128:(mc + 1) * 128],
                          s1_sb[kc][:], start=(kc == 0), stop=(kc == 1))
                  sig = pse.tile([128, EE], BF16, tag="sig", name="sig",
                                 bufs=2)
                  nc.scalar.activation(sig[:], s2_p[:], AF.Sigmoid,
                                       bias=cbb[mc][:], scale=sev["ses2"][mc][:])
                  outv = fusedp[mc][:].rearrange("p (i j) -> p i j",
                                                 j=26)[:, 2:24, 2:24]
                  nc.vector.tensor_tensor(
                      out=outv,
                      in0=fmap[mc][:].rearrange("p (i j) -> p i j", i=E),
                      in1=sig[:].rearrange("p (i j) -> p i j", i=E),
                      op=ALU.mult)

              def keepalive(dep_ap):
                  ka = psse.tile([1, 1], F32, tag="cse", name="ka")
                  nc.tensor.matmul(ka[:], dep_ap, dep_ap, start=True,
                                   stop=True)

              if stages < 4:
                  for mc in range(4):
                      s2_chunk(mc)

              if stages >= 4:
                # ============ stage 4: conv stack ============
                def tap_view(padt, tap):
                    dy, dx = tap // 5, tap % 5
                    return padt[:].rearrange("p (i j) -> p i j",
                                             j=26)[:, dy:dy + 22, dx:dx + 22]

                with tc.tile_pool(name="pcw", bufs=1) as pcw, \
                     tc.tile_pool(name="psc", bufs=1, space="PSUM") as psc:
                    w2 = []
                    for kc in range(2):
                        t = pcw.tile([128, 25 * 128], BF16, tag=f"w2_{kc}",
                                     name=f"w2_{kc}")
                        nc.gpsimd.dma_start(t[:], w2sb_d[kc])
                        w2.append(t)
                    w3 = []
                    for kc in range(2):
                        t = pcw.tile([128, 25 * 256], BF16, tag=f"w3_{kc}",
                                     name=f"w3_{kc}")
                        nc.gpsimd.dma_start(t[:], w3sb_d[kc])
                        w3.append(t)

                    mtop = cf("mtop")
                    mbot = cf("mbot")

                    def interior(t_):
                        return t_[:].rearrange("p (i j) -> p i j",
                                               j=26)[:, 2:24, 2:24]

                    # conv1, interleaved with s2 chunks; keepalives bridge
                    # any DMA-late window at full p-state
                    s2_chunk(0)
                    s2_chunk(1)
                    keepalive(b0sb[0:1, 0:1])
                    keepalive(sewb[0:1, _CS - 1:_CS])
                    r1_p = psc.tile([128, EE], F32, tag="convp", name="convp",
                                    bufs=1)
                    first = True
                    for kc in range(4):
                        if kc + 2 < 4:
                            s2_chunk(kc + 2)
                        if kc + 1 < 4:
                            keepalive(w1[kc][0:1, 0:1])
                        for tap in range(25):
                            nc.tensor.matmul(r1_p[:],
                                             w1[kc][:, tap * 128:(tap + 1) * 128],
                                             tap_view(fusedp[kc], tap),
                                             start=first,
                                             stop=(kc == 3 and tap == 24))
                            first = False
                    r1c = pcw.tile([128, EE], BF16)
                    nc.scalar.activation(r1c[:], r1_p[:], AF.Relu, bias=b1h[:])
                    nc.vector.tensor_copy(out=interior(own1), in_=r1c[:])
                    if dbg == 'r1':
                        dbgt3 = pwork.tile([128, 1024], F32, tag="dbgt",
                                           name="dbgt")
                        nc.vector.tensor_copy(out=dbgt3[:, 0:EE], in_=r1c[:])
                        nc.sync.dma_start(dbg_d[:], dbgt3[:])
                    if dbg == 'fused':
                        dbgt4 = pwork.tile([128, 1024], F32, tag="dbgt",
                                           name="dbgt")
                        nc.vector.tensor_copy(
                            out=dbgt4[:, 0:EE],
                            in_=fusedp[0][:].rearrange("p (i j) -> p i j",
                                                       j=26)[:, 2:24, 2:24])
                        nc.sync.dma_start(dbg_d[:], dbgt4[:])

                    r1b = pdram.tile([128, EE], BF16)
                    nc.sync.dma_start(r1b[:], r1c[:])
                    g1t = pcw.tile([128, EE], BF16)
                    g1b = pcw.tile([128, EE], BF16)
                    if solo:
                        nc.sync.dma_start(g1b[:], r1b[:])
                        nc.sync.dma_start(g1t[:], r1b[:])
                    else:
                        r1g = pdram.tile([256, EE], BF16)
                        nc.gpsimd.collective_compute(
                            "AllGather", ALU.bypass, replica_groups=groups,
                            ins=[r1b[:].opt()], outs=[r1g[:].opt()])
                        nc.sync.dma_start(g1b[:], r1g[128:256, :])
                        nc.sync.dma_start(g1t[:], r1g[0:128, :])
                    tmp1 = pcw.tile([128, EE], F32)
                    nc.vector.tensor_scalar(out=tmp1[:], in0=g1b[:],
                                            scalar1=mbot[:], scalar2=None,
                                            op0=ALU.mult)
                    nc.vector.scalar_tensor_tensor(out=interior(oth1),
                                                   in0=g1t[:], scalar=mtop[:],
                                                   in1=tmp1[:], op0=ALU.mult,
                                                   op1=ALU.add)

                    # conv2: own half first, then other half
                    r2_p = psc.tile([128, EE], F32, tag="convp", name="convp2",
                                    bufs=1)
                    for tap in range(25):
                        nc.tensor.matmul(r2_p[:],
                                         w2[0][:, tap * 128:(tap + 1) * 128],
                                         tap_view(own1, tap),
                                         start=(tap == 0), stop=False)
                    for tap in range(25):
                        nc.tensor.matmul(r2_p[:],
                                         w2[1][:, tap * 128:(tap + 1) * 128],
                                         tap_view(oth1, tap),
                                         start=False, stop=(tap == 24))
                    r2c = pcw.tile([128, EE], BF16)
                    nc.scalar.activation(r2c[:], r2_p[:], AF.Relu, bias=b2h[:])
                    nc.vector.tensor_copy(out=interior(own2), in_=r2c[:])

                    r2b = pdram.tile([128, EE], BF16)
                    nc.sync.dma_start(r2b[:], r2c[:])
                    g2t = pcw.tile([128, EE], BF16)
                    g2b = pcw.tile([128, EE], BF16)
                    if solo:
                        nc.sync.dma_start(g2b[:], r2b[:])
                        nc.sync.dma_start(g2t[:], r2b[:])
                    else:
                        r2g = pdram.tile([256, EE], BF16)
                        nc.gpsimd.collective_compute(
                            "AllGather", ALU.bypass, replica_groups=groups,
                            ins=[r2b[:].opt()], outs=[r2g[:].opt()])
                        nc.sync.dma_start(g2b[:], r2g[128:256, :])
                        nc.sync.dma_start(g2t[:], r2g[0:128, :])
                    tmp2 = pcw.tile([128, EE], F32)
                    nc.vector.tensor_scalar(out=tmp2[:], in0=g2b[:],
                                            scalar1=mbot[:], scalar2=None,
                                            op0=ALU.mult)
                    nc.vector.scalar_tensor_tensor(out=interior(oth2),
                                                   in0=g2t[:], scalar=mtop[:],
                                                   in1=tmp2[:], op0=ALU.mult,
                                                   op1=ALU.add)

                    # conv3: both chunks' own-half taps first (hides r2
                    # gather); oc1 accumulates in two pixel-column halves so
                    # the final relu+store overlaps its last taps
                    r3_ps = [psc.tile([128, EE], F32, tag="convp3",
                                      name="convp3", bufs=3) for _ in range(3)]
                    HALF = 242

                    def half_view(padt, tap, hh):
                        v = tap_view(padt, tap)
                        return v[:, hh * 11:(hh + 1) * 11, :]

                    for tap in range(25):
                        nc.tensor.matmul(
                            r3_ps[0][:], w3[0][:, tap * 256:tap * 256 + 128],
                            tap_view(own2, tap), start=(tap == 0), stop=False)
                    for hh in range(2):
                        for tap in range(25):
                            nc.tensor.matmul(
                                r3_ps[1 + hh][:, 0:HALF],
                                w3[0][:, tap * 256 + 128:tap * 256 + 256],
                                half_view(own2, tap, hh), start=(tap == 0),
                                stop=False)
                    for tap in range(25):
                        nc.tensor.matmul(
                            r3_ps[0][:], w3[1][:, tap * 256:tap * 256 + 128],
                            tap_view(oth2, tap), start=False, stop=(tap == 24))
                    o_sb0 = pcw.tile([128, EE], F32, tag="osb0", name="osb0")
                    nc.scalar.activation(o_sb0[:], r3_ps[0][:], AF.Relu,
                                         bias=b3h[0][:])
                    nc.sync.dma_start(out_d[0:128, :], o_sb0[:])
                    o_sb1 = pcw.tile([128, EE], F32, tag="osb1", name="osb1")
                    for hh in range(2):
                        for tap in range(25):
                            nc.tensor.matmul(
                                r3_ps[1 + hh][:, 0:HALF],
                                w3[1][:, tap * 256 + 128:tap * 256 + 256],
                                half_view(oth2, tap, hh), start=False,
                                stop=(tap == 24))
                        nc.scalar.activation(
                            o_sb1[:, hh * HALF:(hh + 1) * HALF],
                            r3_ps[1 + hh][:, 0:HALF], AF.Relu,
                            bias=b3h[1][:])
                        nc.sync.dma_start(
                            out_d[128:256, hh * HALF:(hh + 1) * HALF],
                            o_sb1[:, hh * HALF:(hh + 1) * HALF])

    nc.compile()
    return nc


_NC_CACHE = None


def _get_program():
    global _NC_CACHE
    if _NC_CACHE is None:
        _NC_CACHE = build_program()
    return _NC_CACHE


def _prep_shared(w):
    """Packed weights/constants identical on every core."""
    ADJ = _build_adj()
    out = {}
    constr = np.zeros((128, _CR), np.float32)
    ctrb = np.zeros((128, _CT), np.float32)

    def put(nm, arr):
        c0, cols = _LAY_R[nm]
        r, cc = arr.shape
        constr[0:r, c0:c0 + cc] = arr

    def putt(nm, arr):
        c0, cols = _LAY_T[nm]
        r, cc = arr.shape
        ctrb[0:r, c0:c0 + cc] = arr
    wt = w['W_trans']
    for kc in range(6):
        putt(f"wtr{kc}", wt[kc * 128:(kc + 1) * 128])
    putt("brow", w['b_trans'].reshape(1, EMB))
    put("onescol", np.ones((128, 1), np.float32))
    putt("onesrow", np.ones((128, 128), np.float32))
    onespad = np.zeros((1, 110), np.float32)
    onespad[0, E:E + EM] = 1.0
    putt("onespad", onespad)
    g2T = np.zeros((110, E), np.float32)
    for e in range(E):
        g2T[E + e * M:E + (e + 1) * M, e] = 1.0
    put("g2T", g2T)
    sumT = np.kron(np.eye(L, dtype=np.float32), np.ones((SPAN, 1), np.float32))
    for kc in range(4):
        sp = np.zeros((128, NN), np.float32)
        sp[:, E + EM:NN] = sumT[kc * 128:(kc + 1) * 128]
        put(f"sumT{kc}", sp)
    out['constr'] = constr
    out['ctrb'] = ctrb.astype(ml_dtypes.bfloat16)

    gT = np.zeros((EMH, E), np.float32)
    for e in range(E):
        gT[e * M * H:(e + 1) * M * H, e] = 1.0 / (M * H)
    gTb = np.zeros((128, 9 * E), np.float32)
    for kc in range(9):
        r = min(128, EMH - kc * 128)
        gTb[0:r, kc * E:(kc + 1) * E] = gT[kc * 128:kc * 128 + r]
    out['gTb'] = gTb.astype(ml_dtypes.bfloat16)
    out['aallTb'] = np.concatenate(
        [ADJ[r].T for r in range(NREL)] + [np.eye(NN, dtype=np.float32)],
        axis=1).astype(ml_dtypes.bfloat16)
    out['identb'] = np.eye(128, dtype=np.float32).astype(ml_dtypes.bfloat16)

    sewb = np.zeros((128, _CS), np.float32)

    def puts(nm, arr):
        c0, cols = _LAY_S[nm]
        sewb[0:arr.shape[0], c0:c0 + arr.shape[1]] = arr
    for nm, arr, nch in (("fsw1T", w['fs_w1'].T, 4), ("fcw1T", w['fc_w1'].T, 4),
                         ("fsw2T", w['fs_w2'].T, 2), ("fcw2T", w['fc_w2'].T, 2)):
        for kc in range(nch):
            puts(f"{nm}{kc}",
                 np.ascontiguousarray(arr[kc * 128:(kc + 1) * 128]))
    out['sewb'] = sewb.astype(ml_dtypes.bfloat16)

    constf = np.zeros((128, _CF), np.float32)

    def putf(nm, arr):
        c0, cols = _LAY_F[nm]
        constf[0:arr.shape[0], c0:c0 + 1] = arr.reshape(-1, 1)
    vecs = {"ses1": w['fs_g1'], "seb1": w['fs_b1'] * w['fs_g1'] + w['fs_be1'],
            "fcs1": w['fc_g1'], "fcb1": w['fc_b1'] * w['fc_g1'] + w['fc_be1'],
            "ses2": w['fs_g2'], "seb2": w['fs_b2'] * w['fs_g2'] + w['fs_be2'],
            "fcs2": w['fc_g2'], "fcb2": w['fc_b2'] * w['fc_g2'] + w['fc_be2']}
    for nm, v in vecs.items():
        nch = 2 if v.shape[0] == INTER else 4
        for kc in range(nch):
            putf(f"{nm}{kc}", v[kc * 128:(kc + 1) * 128])
    out['constf_base'] = constf

    # RGCN weights, r-major pieces; layer-0 type-embed columns folded into B0
    T = np.ascontiguousarray(w['type_embed'][_TYPES])          # [126, 20]
    B0 = T @ w['rgcn_Wself0'][EMB:EMB + TD]
    for r in range(NREL):
        B0 = B0 + ADJ[r] @ (T @ w['rgcn_Wrel0'][r, EMB:EMB + TD])
    out['b0b'] = np.ascontiguousarray(B0).astype(ml_dtypes.bfloat16)
    for layer in range(NLAYERS):
        if layer == 0:
            mats = [w['rgcn_Wrel0'][r, 0:EMB] for r in range(NREL)] + \
                   [w['rgcn_Wself0'][0:EMB]]
        else:
            mats = [w['rgcn_Wrel'][layer - 1, r] for r in range(NREL)] + \
                   [w['rgcn_Wself'][layer - 1]]
        p = np.zeros((128, 16 * EMB), np.float32)
        for r in range(4):
            for si in range(4):
                p[:, (r * 4 + si) * EMB:(r * 4 + si + 1) * EMB] = \
                    mats[r][si * 128:(si + 1) * 128]
        out[f'wstp{layer}'] = p.astype(ml_dtypes.bfloat16)
    return out


def _prep_conv_half(w, half, constf_base):
    out = {}
    w1 = w['cr_w1'][half * 128:(half + 1) * 128]
    out['w1sb'] = np.ascontiguousarray(
        w1.transpose(1, 2, 3, 0).reshape(4, 128, 25 * 128)).astype(
            ml_dtypes.bfloat16)
    w2 = w['cr_w2'][half * 128:(half + 1) * 128]
    w2p = w2.transpose(1, 2, 3, 0).reshape(2, 128, 25 * 128)
    order = [half, 1 - half]
    out['w2sb'] = np.ascontiguousarray(w2p[order]).astype(ml_dtypes.bfloat16)
    w3 = w['cr_w3'][half * 256:(half + 1) * 256]
    w3p = w3.transpose(1, 2, 3, 0).reshape(2, 128, 25 * 256)
    out['w3sb'] = np.ascontiguousarray(w3p[order]).astype(ml_dtypes.bfloat16)
    constf = constf_base.copy()

    def putf(nm, arr):
        c0, cols = _LAY_F[nm]
        constf[0:arr.shape[0], c0:c0 + 1] = arr.reshape(-1, 1)
    putf("b1h", w['cr_b1'][half * 128:(half + 1) * 128])
    putf("b2h", w['cr_b2'][half * 128:(half + 1) * 128])
    putf("b3h0", w['cr_b3'][half * 256:half * 256 + 128])
    putf("b3h1", w['cr_b3'][half * 256 + 128:half * 256 + 256])
    putf("mtop", np.full(128, float(half), np.float32))
    putf("mbot", np.full(128, float(1 - half), np.float32))
    c0, cols = _LAY_F["identf"]
    constf[:, c0:c0 + 128] = np.eye(128, dtype=np.float32)
    out['constf'] = constf
    return out


def _prep_doc(x, att, mi, ls):
    out = {}
    mif = mi.reshape(EM)
    attm = np.ascontiguousarray(
        att[:, mif, :].transpose(1, 0, 2).reshape(EMH, C))
    amp = np.zeros((128, 9 * C), np.float32)
    for kc in range(9):
        r = min(128, EMH - kc * 128)
        amp[0:r, kc * C:kc * C + C] = attm[kc * 128:kc * 128 + r]
    out['amp'] = amp.astype(ml_dtypes.bfloat16)
    idx = ls[:, None] + np.arange(SPAN)
    idxf = idx.reshape(LS)
    rows = att[:, idxf, :].reshape(H, L, SPAN, C)
    blocks = np.take_along_axis(rows, idx[None, :, None, :], axis=3)
    attl = blocks.transpose(0, 2, 1, 3).reshape(HS, LS)
    xmT = x[mif].T
    xspT = x[idxf].T
    actr = np.zeros((128, _CA), np.float32)

    def put(nm, arr):
        c0, cols = _LAY_A[nm]
        actr[0:arr.shape[0], c0:c0 + arr.shape[1]] = arr
    for kc in range(6):
        xmp = np.zeros((128, 110), np.float32)
        xmp[:, E:E + EM] = xmT[kc * 128:(kc + 1) * 128]
        put(f"xmT{kc}", xmp)
        put(f"xspT{kc}", xspT[kc * 128:(kc + 1) * 128])
    out['actr'] = actr.astype(ml_dtypes.bfloat16)
    attb = np.zeros((128, _CB), np.float32)

    def putb(nm, arr):
        c0, cols = _LAY_B[nm]
        attb[0:arr.shape[0], c0:c0 + arr.shape[1]] = arr
    for kc in range(3):
        putb(f"attl{kc}", attl[kc * 128:(kc + 1) * 128])
    putb("onesb", np.ones((128, 1), np.float32))
    out['attb'] = attb.astype(ml_dtypes.bfloat16)
    xpk = np.zeros((128, 8 * HID), np.float32)
    for kc in range(8):
        xpk[:, kc * HID:(kc + 1) * HID] = x[kc * 128:(kc + 1) * 128]
    out['xp'] = xpk.astype(ml_dtypes.bfloat16)
    return out


def build_in_maps(inputs):
    w = {}
    for k, v in inputs.items():
        a = np.asarray(v)
        w[k] = a if a.dtype in (np.int32, np.int64) else \
            np.asarray(a, np.float32)
    shared = _prep_shared(w)
    constf_base = shared.pop('constf_base')
    halves = [_prep_conv_half(w, h, constf_base) for h in range(2)]
    seq = np.asarray(inputs['sequence_output'], np.float32)
    att = np.asarray(inputs['attention'], np.float32)
    mi = np.asarray(inputs['mention_idx']).astype(np.int64)
    ls = np.asarray(inputs['link_start']).astype(np.int64)
    docs = [_prep_doc(seq[n], att[n], mi[n], ls[n]) for n in range(NB)]
    in_maps = []
    for core in range(N_CORES):
        n, half = core // 2, core % 2
        m = dict(shared)
        m.update(halves[half])
        m.update(docs[n])
        in_maps.append({k: (np.ascontiguousarray(v)
                            if v.dtype == ml_dtypes.bfloat16
                            else np.ascontiguousarray(v, np.float32))
                        for k, v in m.items()})
    return in_maps


def kernel(**inputs):
    nc = _get_program()
    in_maps = build_in_maps(inputs)
    res = run_bass_kernel_spmd(nc, in_maps, list(range(N_CORES)))
    out = np.zeros((NB, EMB, E, E), np.float32)
    for core in range(N_CORES):
        n, half = core // 2, core % 2
        out[n, half * 256:(half + 1) * 256] = \
            res.results[core]["out"].reshape(256, E, E)
    return out
